# revision 1
# baseline (speedup 1.0000x reference)
"""Multi-head self-attention TRN2 Bass kernel.

Problem: x[4,2048,512], 8 heads of d=64, scale 1/sqrt(512) (full feature dim).

Sharding: 8 cores = (batch b in 0..3) x (head-group hg in 0..1). Each core
handles one batch element and 4 heads (256 of the 512 features), computing a
partial output projection z_partial = attn_heads @ Wo[hg rows].  The host
sums the two partials per batch and adds bo.

Per-core kernel (all matmuls in float32r at full PE rate):
  prologue: load xT [512,2048] (host pre-transposed), W column/row shards.
            QT/KT = W^T x^T per head-pair [128, 2048] (+bias per partition);
            V = x W per j-tile [128, 256].
  rounds (icb 0..3 i-chunks of 512) x (head pair p 0..1):
    for jt 0..15:
       S^T_h [128j, 512i] = K_h Q_h^T   (two heads row-packed, K=64)
       P^T_h = exp(S^T_h / sqrt(512))   (ScalarE, the bottleneck)
       O^T  += V_h^T P^T_h              (two heads col-packed, M=64)
       rs_h += 1^T P^T_h                (rowsum, col-packed M=1)
    normalize: recip(rs), broadcast via ones-matmul, O^T scaled (DVE)
  epilogue per icb: z^T[f,i] = Wo^T O^T (+ (bv@Wo+?) bias), DMA out zT.

Output zt [512, 2048] = z^T; host transposes back and reduces.
"""

import sys
import os

sys.path.insert(0, "/opt/trn_rl_repo")

import numpy as np

B, N, F = 4, 2048, 512
H, D = 8, 64
P = 128
DH = 256  # features per core (4 heads)
NPAIR = 2  # head pairs per core
KT = F // P  # 4 k-tiles over input features
SCALE = 1.0 / float(np.float32(F) ** 0.5)

_cache = {}


def _bf_np():
    import ml_dtypes

    return np.dtype(ml_dtypes.bfloat16)


def build(n=N, f32r_mm=True):
    """Build + bass-compile the per-core program for sequence length n."""
    import concourse.tile as tile
    from concourse import bacc, mybir

    f32 = mybir.dt.float32
    bf = mybir.dt.bfloat16
    AF = mybir.ActivationFunctionType
    mmdt = mybir.dt.float32r if f32r_mm else f32

    def r(ap):
        return ap

    nt = n // P          # j-tiles
    ic = min(n, 512)     # matmul moving-dim slice
    ICB = min(n, 1024)   # i-chunk per round (ACT op width)
    NH = ICB // ic       # matmul slices per chunk
    nicb = n // ICB
    nic = n // ic

    nc = bacc.Bacc("TRN2", target_bir_lowering=False, debug=False)

    xt_d = nc.dram_tensor("xt", [F, n], mmdt, kind="ExternalInput").ap()
    wq_d = nc.dram_tensor("wq", [F, DH], mmdt, kind="ExternalInput").ap()
    wk_d = nc.dram_tensor("wk", [F, DH], mmdt, kind="ExternalInput").ap()
    wv_d = nc.dram_tensor("wv", [F, DH], mmdt, kind="ExternalInput").ap()
    wo_d = nc.dram_tensor("wo", [DH, F], mmdt, kind="ExternalInput").ap()
    bq_d = nc.dram_tensor("bq2", [P, NPAIR], f32, kind="ExternalInput").ap()
    bk_d = nc.dram_tensor("bk2", [P, NPAIR], f32, kind="ExternalInput").ap()
    zb_d = nc.dram_tensor("zb4", [P, F // P], f32, kind="ExternalInput").ap()
    ones_rs_d = nc.dram_tensor("ones_rs", [P, 1], bf, kind="ExternalInput").ap()
    ones_bc_d = nc.dram_tensor("ones_bc", [1, D], bf, kind="ExternalInput").ap()
    zt_d = nc.dram_tensor("zt", [F, n], f32, kind="ExternalOutput").ap()

    from contextlib import ExitStack

    with tile.TileContext(nc) as tc, ExitStack() as ctx:
        const = ctx.enter_context(tc.tile_pool(name="const", bufs=1))
        pt_pool = ctx.enter_context(tc.tile_pool(name="pt", bufs=6))
        rcp_pool = ctx.enter_context(tc.tile_pool(name="rcp", bufs=2))
        bc_pool = ctx.enter_context(tc.tile_pool(name="bc", bufs=2))
        zts_pool = ctx.enter_context(tc.tile_pool(name="zts", bufs=4))
        ps_s = ctx.enter_context(tc.tile_pool(name="ps_s", bufs=2, space="PSUM"))
        ps_pv = ctx.enter_context(tc.tile_pool(name="ps_pv", bufs=1, space="PSUM"))
        ps_rs = ctx.enter_context(tc.tile_pool(name="ps_rs", bufs=1, space="PSUM"))

        # ---- constant loads -------------------------------------------------
        xt = [const.tile([P, n], mmdt, tag=f"xt{k}", name=f"xt{k}") for k in range(KT)]
        nh2 = max(n // 2, 1)
        for half in range(2 if n > 1 else 1):
            hs = slice(half * nh2, (half + 1) * nh2)
            for k in range(KT):
                nc.sync.dma_start(xt[k][:, hs], xt_d[k * P:(k + 1) * P, hs])
        wq = [const.tile([P, DH], mmdt, tag=f"wq{k}", name=f"wq{k}") for k in range(KT)]
        wk = [const.tile([P, DH], mmdt, tag=f"wk{k}", name=f"wk{k}") for k in range(KT)]
        wv = [const.tile([P, DH], mmdt, tag=f"wv{k}", name=f"wv{k}") for k in range(KT)]
        for k in range(KT):
            nc.sync.dma_start(wk[k][:], wk_d[k * P:(k + 1) * P, :])
            nc.sync.dma_start(wq[k][:], wq_d[k * P:(k + 1) * P, :])
            nc.sync.dma_start(wv[k][:], wv_d[k * P:(k + 1) * P, :])
        wo = [const.tile([P, F], mmdt, tag=f"wo{k}", name=f"wo{k}") for k in range(DH // P)]
        for k in range(DH // P):
            nc.sync.dma_start(wo[k][:], wo_d[k * P:(k + 1) * P, :])
        bq_sb = const.tile([P, NPAIR], f32, tag="bq", name="bq_sb")
        bk_sb = const.tile([P, NPAIR], f32, tag="bk", name="bk_sb")
        zb_sb = const.tile([P, F // P], f32, tag="zb", name="zb_sb")
        nc.sync.dma_start(bq_sb[:], bq_d[:])
        nc.sync.dma_start(bk_sb[:], bk_d[:])
        nc.sync.dma_start(zb_sb[:], zb_d[:])
        ones_rs = const.tile([P, 1], bf, tag="ones_rs", name="ones_rs")
        nc.sync.dma_start(ones_rs[:], ones_rs_d[:])
        ones_bc = const.tile([1, D], bf, tag="ones_bc", name="ones_bc")
        nc.sync.dma_start(ones_bc[:], ones_bc_d[:])

        # warm the exp table set on ScalarE while DMAs stream in
        warm = const.tile([1, 1], f32, tag="warm", name="warm")
        nc.vector.memset(warm[:], 0.0)
        nc.scalar.activation(warm[:], warm[:], AF.Exp)

        # persistent activations
        qt = [const.tile([P, n], mmdt, tag=f"qt{p}", name=f"qt{p}") for p in range(NPAIR)]
        kt_sb = [const.tile([P, n], mmdt, tag=f"kt{p}", name=f"ktsb{p}") for p in range(NPAIR)]
        v_sb = [const.tile([P, DH], bf, tag=f"v{j}", name=f"v{j}") for j in range(nt)]
        ot = [const.tile([P, n], mmdt, tag=f"ot{p}", name=f"ot{p}") for p in range(NPAIR)]

        # ---- QKV projections -----------------------------------------------
        _prol_ctr = [0]

        def _prol_ps(shape):
            _prol_ctr[0] += 1
            pool, tg = ((ps_pv, "pv"), (ps_rs, "rs"))[_prol_ctr[0] % 2]
            return pool.tile(shape, f32, tag=tg, name=tg)

        def proj_qk(p, w_t, b_sb, dst, ib, late=False):
            ps = ps_s.tile([P, ICB], f32, tag="st", name="st") if late else _prol_ps([P, ICB])
            for i5 in range(NH):
                for k in range(KT):
                    nc.tensor.matmul(
                        ps[:, i5 * ic:(i5 + 1) * ic],
                        w_t[k][:, p * P:(p + 1) * P],
                        xt[k][:, (ib * NH + i5) * ic:(ib * NH + i5 + 1) * ic],
                        start=(k == 0),
                        stop=(k == KT - 1),
                    )
            nc.vector.tensor_scalar_add(
                dst[p][:, ib * ICB:(ib + 1) * ICB], ps[:], b_sb[:, p:p + 1]
            )

        def proj_v(j, late=False):
            ps = ps_s.tile([P, DH], f32, tag="st", name="st") if late else _prol_ps([P, DH])
            for k in range(KT):
                nc.tensor.matmul(
                    ps[:],
                    xt[k][:, j * P:(j + 1) * P],
                    wv[k][:],
                    start=(k == 0),
                    stop=(k == KT - 1),
                )
            nc.vector.tensor_copy(v_sb[j][:], ps[:])

        # emission order: only what round (icb0, p0) jt0 needs up front;
        # V and remaining Q/K projections interleave into round 0's jt loop.
        proj_qk(0, wk, bk_sb, kt_sb, 0)
        proj_qk(0, wq, bq_sb, qt, 0)
        proj_v(0)
        proj_v(1)
        late_work = [lambda ib=ib: proj_qk(0, wk, bk_sb, kt_sb, ib, late=True)
                     for ib in range(1, nicb)]
        late_work += [lambda j=j: proj_v(j, late=True) for j in range(2, nt)]
        late_work += [lambda ib=ib: proj_qk(0, wq, bq_sb, qt, ib, late=True) for ib in range(1, nicb)]
        late_work += [lambda ib=ib: proj_qk(1, wk, bk_sb, kt_sb, ib, late=True) for ib in range(nicb)]
        late_work += [lambda ib=ib: proj_qk(1, wq, bq_sb, qt, ib, late=True) for ib in range(nicb)]

        # ---- attention rounds ----------------------------------------------
        deferred_z = []

        def z_proj(icb, ft):
            zps = ps_s.tile([P, ICB], f32, tag="st", name="st")
            for i5 in range(NH):
                i5s = slice(i5 * ic, (i5 + 1) * ic)
                for k in range(DH // P):
                    nc.tensor.matmul(
                        zps[:, i5s],
                        wo[k][:, ft * P:(ft + 1) * P],
                        ot[k][:, (icb * NH + i5) * ic:(icb * NH + i5 + 1) * ic],
                        start=(k == 0),
                        stop=(k == DH // P - 1),
                    )
            zsb = zts_pool.tile([P, ICB], f32, tag="zt", name="ztsb")
            nc.vector.tensor_scalar_add(zsb[:], zps[:], zb_sb[:, ft:ft + 1])
            nc.sync.dma_start(
                zt_d[ft * P:(ft + 1) * P, icb * ICB:(icb + 1) * ICB], zsb[:]
            )

        for icb in range(nicb):
            isl = slice(icb * ICB, (icb + 1) * ICB)
            for p in range(NPAIR):
                while late_work and not (icb == 0 and p == 0):
                    late_work.pop(0)()
                pvps = ps_pv.tile([P, ICB], f32, tag="pv", name="pv")
                rsps = ps_rs.tile([P, ICB], f32, tag="rs", name="rs")
                for jt in range(nt):
                    sps = [ps_s.tile([P, ICB], f32, tag="st", name="st") for _ in range(2)]
                    # S^T (row-packed head pair, K=64 each)
                    for i5 in range(NH):
                        for h in range(2):
                            hp = slice(64 * h, 64 * (h + 1))
                            nc.tensor.matmul(
                                sps[h][:, i5 * ic:(i5 + 1) * ic],
                                kt_sb[p][hp, jt * P:(jt + 1) * P],
                                qt[p][hp, (icb * NH + i5) * ic:(icb * NH + i5 + 1) * ic],
                                start=True,
                                stop=True,
                                tile_position=(64 * h, 0),
                            )
                    # interleave leftover projections into round 0's stream
                    if icb == 0 and p == 0 and jt >= 1:
                        for _ in range(2):
                            if late_work:
                                late_work.pop(0)()
                    elif deferred_z and jt >= 1 and jt % 2 == 0:
                        deferred_z.pop(0)()
                    # exp (ScalarE, the bottleneck) -> P^T bf16 in SBUF
                    pts = []
                    for h in range(2):
                        ptile = pt_pool.tile([P, ICB], bf, tag="pt", name="pt")
                        nc.scalar.activation(ptile[:], sps[h][:], AF.Exp, scale=SCALE)
                        pts.append(ptile)
                    # PV (col-packed pair, M=64) + rowsum (col-packed, M=1)
                    for i5 in range(NH):
                        i5s = slice(i5 * ic, (i5 + 1) * ic)
                        for h in range(2):
                            hl = 2 * p + h
                            nc.tensor.matmul(
                                pvps[64 * h:64 * (h + 1), i5s],
                                v_sb[jt][:, 64 * hl:64 * (hl + 1)],
                                pts[h][:, i5s],
                                start=(jt == 0),
                                stop=(jt == nt - 1),
                                tile_position=(0, 64 * h),
                                skip_group_check=True,
                            )
                    for i5 in range(NH):
                        i5s = slice(i5 * ic, (i5 + 1) * ic)
                        for h in range(2):
                            nc.tensor.matmul(
                                rsps[32 * h:32 * h + 1, i5s],
                                ones_rs[:],
                                pts[h][:, i5s],
                                start=(jt == 0),
                                stop=(jt == nt - 1),
                                tile_position=(0, 32 * h),
                                skip_group_check=True,
                            )
                # normalize: 1/rs broadcast via ones outer-product, O^T * bc
                rcps = []
                for h in range(2):
                    rc = rcp_pool.tile([1, ICB], bf, tag=f"rcp{h}", name=f"rcp{h}")
                    with nc.allow_low_precision(reason="rowsum reciprocal bf16"):
                        nc.vector.reciprocal(rc[:], rsps[32 * h:32 * h + 1, :])
                    rcps.append(rc)
                bcps = ps_rs.tile([P, ICB], f32, tag="rs", name="rs")
                for i5 in range(NH):
                    i5s = slice(i5 * ic, (i5 + 1) * ic)
                    for h in range(2):
                        nc.tensor.matmul(
                            bcps[64 * h:64 * (h + 1), i5s],
                            ones_bc[:],
                            rcps[h][:, i5s],
                            start=True,
                            stop=True,
                            tile_position=(0, 64 * h),
                            skip_group_check=True,
                        )
                bc_sb = bc_pool.tile([P, ICB], f32, tag="bc", name="bc_sb")
                nc.vector.tensor_copy(bc_sb[:], bcps[:])
                nc.vector.tensor_mul(ot[p][:, isl], pvps[:], bc_sb[:])

            # ---- output projection for this i-chunk: deferred, drained
            # inside the next round's jt stream (tail only for the last icb)
            deferred_z += [lambda icb=icb, ft=ft: z_proj(icb, ft)
                           for ft in range(F // P)]

        while deferred_z:
            deferred_z.pop(0)()

    nc.compile()
    return nc


def _get_nc(n=N, f32r_mm=True):
    key = (n, f32r_mm)
    if key not in _cache:
        _cache[key] = build(n, f32r_mm)
    return _cache[key]


def make_in_maps(x, Wq, bq, Wk, bk, Wv, bv, Wo, bo, n=N):
    """Host-side sharding: per-core input dict for core c = 2*b + hg."""
    in_maps = []
    for c in range(8):
        b, hg = divmod(c, 2)
        cs = slice(hg * DH, (hg + 1) * DH)
        wo_s = np.ascontiguousarray(Wo[cs, :])
        zb = np.asarray(bv[cs] @ wo_s, dtype=np.float32)
        in_maps.append({
            "xt": np.ascontiguousarray(np.asarray(x[b]).T),
            "wq": np.ascontiguousarray(Wq[:, cs]),
            "wk": np.ascontiguousarray(Wk[:, cs]),
            "wv": np.ascontiguousarray(Wv[:, cs]),
            "wo": wo_s,
            "bq2": np.ascontiguousarray(np.asarray(bq[cs]).reshape(NPAIR, P).T),
            "bk2": np.ascontiguousarray(np.asarray(bk[cs]).reshape(NPAIR, P).T),
            "zb4": np.ascontiguousarray(zb.reshape(F // P, P).T),
            "ones_rs": np.ones((P, 1), dtype=_bf_np()),
            "ones_bc": np.ones((1, D), dtype=_bf_np()),
        })
    return in_maps


def kernel(x, Wq, bq, Wk, bk, Wv, bv, Wo, bo):
    from concourse.bass_utils import run_bass_kernel_spmd

    x = np.asarray(x, dtype=np.float32)
    args = [np.asarray(a, dtype=np.float32) for a in (Wq, bq, Wk, bk, Wv, bv, Wo, bo)]
    nc = _get_nc()
    in_maps = make_in_maps(x, *args)
    res = run_bass_kernel_spmd(nc, in_maps, list(range(8)))
    bo = args[-1]
    out = np.empty((B, N, F), dtype=np.float32)
    for b in range(B):
        zt0 = res.results[2 * b]["zt"]
        zt1 = res.results[2 * b + 1]["zt"]
        out[b] = (zt0 + zt1).T + bo
    return out



# revision 9
# speedup vs baseline: 1.3994x; 1.3994x over previous
"""Multi-head self-attention TRN2 Bass kernel.

Problem: x[4,2048,512], 8 heads of d=64, scale 1/sqrt(512) (full feature dim).

Sharding: 8 cores = (batch b in 0..3) x (head-group hg in 0..1). Each core
handles one batch element and 4 heads (256 of the 512 features), computing a
partial output projection z_partial = attn_heads @ Wo[hg rows].  The host
sums the two partials per batch and adds bo.

Per-core dataflow (ACT exp is the bottleneck; PE work minimized):
  prologue: xt [512,2048] (host pre-transposed) streamed in; QT/KT = W^T x^T
            per head-pair [128, n] (+bias per partition); V per j-tile
            [128, 260] bf16 with a ones column per head (col 64 of each 65).
  rounds (icb 0..1 i-chunks of 1024) x (head pair p 0..1), jt 0..15:
    S^T_h [128j, 1024i] = K_h Q_h^T  (two heads row-packed via tile_position)
    P^T_h = exp(S^T_h / sqrt(512))   (ScalarE -> bf16 SBUF)
    O[i_sub, 65] += P^T_slice.T V_ext  (lhsT = P^T 128x128 slice, rhs = V_ext
       [128, 65]; col 64 accumulates the softmax row-sum -> per-partition!)
  normalize: rcp = 1/O[:,64] (DVE [128,1]); O_norm = O[:, :64] * rcp
    (tensor_scalar, per-partition broadcast) -> staged [128 i, 128 d] bf16.
  O^T via DMA xbar transpose (SP-issued, idle DMA engines) -> ot[kt][128d, n].
  z^T[f,i] = Wo^T O^T (bf16) + bias, DMA out per 512-chunk.

Emission is software-pipelined so ACT never waits: S(jt+1) is emitted before
PV(jt); projections and z chunks fill PE idle windows from a queue.

Output zt [512, 2048] = z^T; host transposes back, sums partials, adds bo.
"""

import sys
import os

sys.path.insert(0, "/opt/trn_rl_repo")

import numpy as np

B, N, F = 4, 2048, 512
H, D = 8, 64
P = 128
DH = 256   # features per core (4 heads)
NPAIR = 2  # head pairs per core
KT = F // P          # 4 k-tiles over input features
ICB = 1024           # i-chunk per round
NICB = N // ICB      # 2
NT = N // P          # 16 j-tiles
NSUB = ICB // P      # 8 i-subtiles per chunk
SCALE = 1.0 / float(np.float32(F) ** 0.5)

_cache = {}


def _bf_np():
    import ml_dtypes

    return np.dtype(ml_dtypes.bfloat16)


def build():
    """Build + bass-compile the per-core program."""
    import concourse.tile as tile
    from concourse import bacc, mybir
    from contextlib import ExitStack

    f32 = mybir.dt.float32
    f32r = mybir.dt.float32r
    bf = mybir.dt.bfloat16
    AF = mybir.ActivationFunctionType

    n = N
    nc = bacc.Bacc("TRN2", target_bir_lowering=False, debug=False)

    xt_d = nc.dram_tensor("xt", [F, n], f32r, kind="ExternalInput").ap()
    wq_d = nc.dram_tensor("wq", [F, DH], f32r, kind="ExternalInput").ap()
    wk_d = nc.dram_tensor("wk", [F, DH], f32r, kind="ExternalInput").ap()
    wv_d = nc.dram_tensor("wv", [F, DH], f32r, kind="ExternalInput").ap()
    wo_d = nc.dram_tensor("wob", [DH, F], bf, kind="ExternalInput").ap()
    bq_d = nc.dram_tensor("bq2", [P, NPAIR], f32, kind="ExternalInput").ap()
    bk_d = nc.dram_tensor("bk2", [P, NPAIR], f32, kind="ExternalInput").ap()
    zb_d = nc.dram_tensor("zb4", [P, F // P], f32, kind="ExternalInput").ap()
    id_d = nc.dram_tensor("ident", [P, P], bf, kind="ExternalInput").ap()
    zt_d = nc.dram_tensor("zt", [F, n], f32, kind="ExternalOutput").ap()

    from contextlib import ExitStack

    with tile.TileContext(nc) as tc, ExitStack() as ctx:
        const = ctx.enter_context(tc.tile_pool(name="const", bufs=1))
        pt_pool = ctx.enter_context(tc.tile_pool(name="pt", bufs=6))
        rc_pool = ctx.enter_context(tc.tile_pool(name="rc", bufs=8))
        on_pool = ctx.enter_context(tc.tile_pool(name="on", bufs=16))
        zs_pool = ctx.enter_context(tc.tile_pool(name="zs", bufs=4))
        ps_s = ctx.enter_context(tc.tile_pool(name="ps_s", bufs=2, space="PSUM"))
        po_pool = ctx.enter_context(tc.tile_pool(name="po", bufs=3, space="PSUM"))
        pz_pool = ctx.enter_context(tc.tile_pool(name="pz", bufs=1, space="PSUM"))

        # ---- DMA loads (ordered: first-needed first) ------------------------
        xt = [const.tile([P, n], f32r, tag=f"xt{k}", name=f"xt{k}") for k in range(KT)]
        for k in range(KT):
            nc.sync.dma_start(xt[k][:, 0:ICB], xt_d[k * P:(k + 1) * P, 0:ICB])
        wk = [const.tile([P, DH], f32r, tag=f"wk{k}", name=f"wk{k}") for k in range(KT)]
        wq = [const.tile([P, DH], f32r, tag=f"wq{k}", name=f"wq{k}") for k in range(KT)]
        wv = [const.tile([P, DH], f32r, tag=f"wv{k}", name=f"wv{k}") for k in range(KT)]
        for k in range(KT):
            nc.sync.dma_start(wk[k][:], wk_d[k * P:(k + 1) * P, :])
        for k in range(KT):
            nc.sync.dma_start(wq[k][:], wq_d[k * P:(k + 1) * P, :])
        bq_sb = const.tile([P, NPAIR], f32, tag="bq", name="bq_sb")
        bk_sb = const.tile([P, NPAIR], f32, tag="bk", name="bk_sb")
        zb_sb = const.tile([P, F // P], f32, tag="zb", name="zb_sb")
        nc.sync.dma_start(bk_sb[:], bk_d[:])
        nc.sync.dma_start(bq_sb[:], bq_d[:])
        for k in range(KT):
            nc.sync.dma_start(wv[k][:], wv_d[k * P:(k + 1) * P, :])
        nc.sync.dma_start(zb_sb[:], zb_d[:])
        wo = [const.tile([P, F], bf, tag=f"wo{k}", name=f"wo{k}") for k in range(DH // P)]
        for k in range(DH // P):
            nc.sync.dma_start(wo[k][:], wo_d[k * P:(k + 1) * P, :])
        for k in range(KT):
            nc.sync.dma_start(xt[k][:, ICB:n], xt_d[k * P:(k + 1) * P, ICB:n])

        ident = const.tile([P, P], bf, tag="ident", name="ident")
        nc.sync.dma_start(ident[:], id_d[:])

        # warm the exp table set on ScalarE while DMAs stream in
        warm = const.tile([1, 1], f32, tag="warm", name="warm")
        nc.vector.memset(warm[:], 0.0)
        nc.scalar.activation(warm[:], warm[:], AF.Exp)

        # persistent activations
        qt = [const.tile([P, n], f32r, tag=f"qt{p}", name=f"qt{p}") for p in range(NPAIR)]
        kt_sb = [const.tile([P, n], f32r, tag=f"kt{p}", name=f"ktsb{p}") for p in range(NPAIR)]
        # V per j-tile: [128, 260] bf16, head hl at cols [65*hl, 65*hl+64),
        # ones at col 65*hl+64 (accumulates softmax row-sums in PV).
        v_sb = [const.tile([P, 4 * (D + 1)], bf, tag=f"v{j}", name=f"v{j}")
                for j in range(NT)]
        for j in range(NT):
            nc.gpsimd.memset(v_sb[j][:], 1.0)
        # O^T staging for the z projection: [128 d, n] bf16 per k-tile (pair)
        ot = [const.tile([P, n], bf, tag=f"ot{p}", name=f"ot{p}") for p in range(NPAIR)]

        # ---- projections ----------------------------------------------------
        def proj_qk_half(p, w_t, b_sb, dst, ib, half, pool):
            """One 512-wide half of a Q/K projection chunk."""
            ps = pool.tile([P, 512], f32, tag="pz", name="pjh") if pool is pz_pool \
                else pool.tile([P, ICB], f32, tag="st", name="pjs")
            psl = ps[:, 0:512]
            isl = slice(ib * ICB + half * 512, ib * ICB + (half + 1) * 512)
            for k in range(KT):
                nc.tensor.matmul(
                    psl,
                    w_t[k][:, p * P:(p + 1) * P],
                    xt[k][:, isl],
                    start=(k == 0),
                    stop=(k == KT - 1),
                )
            nc.vector.tensor_scalar_add(dst[p][:, isl], psl, b_sb[:, p:p + 1])

        def proj_v(j, pool):
            """V for j-tile j: matmul [128,256] then interleave into v_sb."""
            ps = pool.tile([P, 512], f32, tag="pz", name="pvh") if pool is pz_pool \
                else pool.tile([P, ICB], f32, tag="st", name="pvs")
            psl = ps[:, 0:DH]
            for k in range(KT):
                nc.tensor.matmul(
                    psl,
                    xt[k][:, j * P:(j + 1) * P],
                    wv[k][:],
                    start=(k == 0),
                    stop=(k == KT - 1),
                )
            for hl in range(4):
                nc.vector.tensor_copy(
                    v_sb[j][:, hl * (D + 1):hl * (D + 1) + D],
                    psl[:, hl * D:(hl + 1) * D],
                )

        # prologue projections (ps_s is free before the rounds start)
        for half in range(2):
            proj_qk_half(0, wk, bk_sb, kt_sb, 0, half, ps_s)
        for half in range(2):
            proj_qk_half(0, wq, bq_sb, qt, 0, half, ps_s)
        proj_v(0, pz_pool)
        proj_v(1, pz_pool)

        # late work queue: emitted into PE idle windows inside the jt loops
        late_work = []
        late_work += [lambda h=h: proj_qk_half(0, wk, bk_sb, kt_sb, 1, h, pz_pool)
                      for h in range(2)]
        late_work += [lambda j=j: proj_v(j, pz_pool) for j in range(2, 6)]
        late_work += [lambda h=h: proj_qk_half(1, wk, bk_sb, kt_sb, 0, h, pz_pool)
                      for h in range(2)]
        late_work += [lambda j=j: proj_v(j, pz_pool) for j in range(6, 10)]
        late_work += [lambda h=h: proj_qk_half(1, wk, bk_sb, kt_sb, 1, h, pz_pool)
                      for h in range(2)]
        late_work += [lambda j=j: proj_v(j, pz_pool) for j in range(10, 14)]
        late_work += [lambda h=h: proj_qk_half(1, wq, bq_sb, qt, 0, h, pz_pool)
                      for h in range(2)]
        late_work += [lambda j=j: proj_v(j, pz_pool) for j in range(14, 16)]
        # needed from round 2 on
        late_work += [lambda h=h: proj_qk_half(0, wq, bq_sb, qt, 1, h, pz_pool)
                      for h in range(2)]
        late_work += [lambda h=h: proj_qk_half(1, wq, bq_sb, qt, 1, h, pz_pool)
                      for h in range(2)]

        def z_chunk(icb, ft, ch):
            """z^T[ft*128:(ft+1)*128, 512-chunk ch of icb] via pz bank."""
            zp = pz_pool.tile([P, 512], f32, tag="pz", name="zp")
            isl = slice(icb * ICB + ch * 512, icb * ICB + (ch + 1) * 512)
            for k in range(DH // P):
                nc.tensor.matmul(
                    zp[:],
                    wo[k][:, ft * P:(ft + 1) * P],
                    ot[k][:, isl],
                    start=(k == 0),
                    stop=(k == DH // P - 1),
                )
            zsb = zs_pool.tile([P, 512], f32, tag="zt", name="zsb")
            nc.vector.tensor_scalar_add(zsb[:], zp[:], zb_sb[:, ft:ft + 1])
            nc.sync.dma_start(zt_d[ft * P:(ft + 1) * P, isl], zsb[:])

        # ---- attention rounds ----------------------------------------------
        def ob_slice(ob, h, sub):
            """PSUM accumulator slice [128, 65] for (head h, i-subtile sub)."""
            if sub < 7:
                t = ob[h]
                c0 = sub * (D + 1)
            else:
                t = ob[2]
                c0 = h * (D + 1)
            return t[:, c0:c0 + D + 1]

        def emit_S(p, icb, jt, h):
            sps = ps_s.tile([P, ICB], f32, tag="st", name="sps")
            hp = slice(D * h, D * (h + 1))
            for i5 in range(2):
                isl = slice(icb * ICB + i5 * 512, icb * ICB + (i5 + 1) * 512)
                nc.tensor.matmul(
                    sps[:, i5 * 512:(i5 + 1) * 512],
                    kt_sb[p][hp, jt * P:(jt + 1) * P],
                    qt[p][hp, isl],
                    start=True,
                    stop=True,
                    tile_position=(D * h, 0),
                )
            return sps

        rounds = [(0, 0), (0, 1), (1, 0), (1, 1)]
        zq = []  # deferred z chunks

        for r, (icb, p) in enumerate(rounds):
            ob = [po_pool.tile([P, 512], f32, tag="po", name=f"ob{i}")
                  for i in range(3)]
            sps_tiles = {}
            sps_tiles[0] = emit_S(p, icb, 0, 0)
            sps_tiles[1] = emit_S(p, icb, 0, 1)
            for jt in range(NT):
                # prefetch next jt's S while this jt's exp runs (PE stays in
                # front of ACT); fillers go into the PE idle windows
                nxt = {}
                if jt + 1 < NT:
                    nxt[0] = emit_S(p, icb, jt + 1, 0)
                if late_work:
                    late_work.pop(0)()
                elif zq and r >= 2 and jt % 2 == 0:
                    zq.pop(0)()
                if jt + 1 < NT:
                    nxt[1] = emit_S(p, icb, jt + 1, 1)
                if late_work:
                    late_work.pop(0)()
                elif zq and r >= 2 and jt % 2 == 1:
                    zq.pop(0)()
                for h in range(2):
                    hl = 2 * p + h
                    ptile = pt_pool.tile([P, ICB], bf, tag="pt", name="pt")
                    nc.scalar.activation(ptile[:], sps_tiles[h][:], AF.Exp, scale=SCALE)
                    for sub in range(NSUB):
                        # start=True zeroes the whole 2KB psum bank row, so
                        # only the first matmul into each ob bank may set it
                        first_in_bank = (sub == 0) or (sub == 7 and h == 0)
                        nc.tensor.matmul(
                            ob_slice(ob, h, sub),
                            ptile[:, sub * P:(sub + 1) * P],
                            v_sb[jt][:, hl * (D + 1):(hl + 1) * (D + 1)],
                            start=(jt == 0 and first_in_bank),
                            stop=(jt == NT - 1),
                            skip_group_check=True,
                        )
                sps_tiles = nxt

            # ---- normalize + stage + transpose (frees the po bank slices) --
            onst = [on_pool.tile([P, P], bf, tag="on", name=f"on{s}")
                    for s in range(NSUB)]
            for h in range(2):
                for sub in range(NSUB):
                    sl = ob_slice(ob, h, sub)
                    rc = rc_pool.tile([P, 1], f32, tag="rc", name="rc")
                    nc.vector.reciprocal(rc[:], sl[:, D:D + 1])
                    nc.vector.tensor_scalar_mul(
                        onst[sub][:, h * D:(h + 1) * D], sl[:, 0:D], rc[:]
                    )
            # transpose O_norm [128 i, 64 d] -> [64 d, 128 i] via identity
            # matmul (out = O_norm.T @ I), 4 per pz bank tile, then copy into
            # the persistent ot tiles (h0 -> DVE, h1 -> Pool to split load)
            items = [(h, sub) for h in range(2) for sub in range(NSUB)]
            for g in range(4):
                grp = items[g * 4:(g + 1) * 4]
                tp = pz_pool.tile([P, 512], f32, tag="pz", name="tp")
                for i, (h, sub) in enumerate(grp):
                    nc.tensor.matmul(
                        tp[0:D, i * P:(i + 1) * P],
                        onst[sub][:, h * D:(h + 1) * D],
                        ident[:],
                        start=(i == 0),
                        stop=(i == 3),
                        skip_group_check=True,
                    )
                for i, (h, sub) in enumerate(grp):
                    dst = ot[p][h * D:(h + 1) * D,
                                icb * ICB + sub * P: icb * ICB + (sub + 1) * P]
                    nc.vector.tensor_copy(dst, tp[0:D, i * P:(i + 1) * P])
            if p == 1:
                zq += [lambda icb=icb, ft=ft, ch=ch: z_chunk(icb, ft, ch)
                       for ft in range(F // P) for ch in range(ICB // 512)]

        # ---- tail: z chunks for icb=1 via the freed ps_s pool --------------
        # (zq lambdas carry defaults (icb, ft, ch))
        tail = [fn.__defaults__ for fn in zq]
        zq = []
        for ti in range(0, len(tail), 2):
            zp = ps_s.tile([P, ICB], f32, tag="st", name="zptail")
            zsb = zs_pool.tile([P, ICB], f32, tag="ztw", name="zsbw")
            grp = tail[ti:ti + 2]
            for gi, (icb, ft, ch) in enumerate(grp):
                isl = slice(icb * ICB + ch * 512, icb * ICB + (ch + 1) * 512)
                for k in range(DH // P):
                    nc.tensor.matmul(
                        zp[:, gi * 512:(gi + 1) * 512],
                        wo[k][:, ft * P:(ft + 1) * P],
                        ot[k][:, isl],
                        start=(k == 0),
                        stop=(k == DH // P - 1),
                    )
            for gi, (icb, ft, ch) in enumerate(grp):
                isl = slice(icb * ICB + ch * 512, icb * ICB + (ch + 1) * 512)
                nc.vector.tensor_scalar_add(
                    zsb[:, gi * 512:(gi + 1) * 512],
                    zp[:, gi * 512:(gi + 1) * 512],
                    zb_sb[:, ft:ft + 1],
                )
                nc.sync.dma_start(
                    zt_d[ft * P:(ft + 1) * P, isl],
                    zsb[:, gi * 512:(gi + 1) * 512],
                )

    nc.compile()
    return nc


def _get_nc():
    if "nc" not in _cache:
        _cache["nc"] = build()
    return _cache["nc"]


def make_in_maps(x, Wq, bq, Wk, bk, Wv, bv, Wo, bo):
    """Host-side sharding: per-core input dict for core c = 2*b + hg."""
    bfnp = _bf_np()
    in_maps = []
    for c in range(8):
        b, hg = divmod(c, 2)
        cs = slice(hg * DH, (hg + 1) * DH)
        wo_s = np.ascontiguousarray(Wo[cs, :])
        zb = np.asarray(bv[cs] @ wo_s, dtype=np.float32)
        in_maps.append({
            "xt": np.ascontiguousarray(np.asarray(x[b]).T),
            "wq": np.ascontiguousarray(Wq[:, cs]),
            "wk": np.ascontiguousarray(Wk[:, cs]),
            "wv": np.ascontiguousarray(Wv[:, cs]),
            "wob": np.ascontiguousarray(wo_s.astype(bfnp)),
            "bq2": np.ascontiguousarray(np.asarray(bq[cs]).reshape(NPAIR, P).T),
            "bk2": np.ascontiguousarray(np.asarray(bk[cs]).reshape(NPAIR, P).T),
            "zb4": np.ascontiguousarray(zb.reshape(F // P, P).T),
            "ident": np.eye(P, dtype=bfnp),
        })
    return in_maps


def kernel(x, Wq, bq, Wk, bk, Wv, bv, Wo, bo):
    from concourse.bass_utils import run_bass_kernel_spmd

    x = np.asarray(x, dtype=np.float32)
    args = [np.asarray(a, dtype=np.float32) for a in (Wq, bq, Wk, bk, Wv, bv, Wo, bo)]
    nc = _get_nc()
    in_maps = make_in_maps(x, *args)
    res = run_bass_kernel_spmd(nc, in_maps, list(range(8)))
    bo = args[-1]
    out = np.empty((B, N, F), dtype=np.float32)
    for b in range(B):
        zt0 = res.results[2 * b]["zt"]
        zt1 = res.results[2 * b + 1]["zt"]
        out[b] = (zt0 + zt1).T + bo
    return out


# revision 12
# speedup vs baseline: 1.4828x; 1.0596x over previous
"""Multi-head self-attention TRN2 Bass kernel.

Problem: x[4,2048,512], 8 heads of d=64, scale 1/sqrt(512) (full feature dim).

Sharding: 8 cores = (batch b in 0..3) x (head-group hg in 0..1). Each core
handles one batch element and 4 heads (256 of the 512 features), computing a
partial output projection z_partial = attn_heads @ Wo[hg rows].  The host
sums the two partials per batch and adds bo.

Per-core dataflow (ACT exp is the bottleneck; PE work minimized):
  prologue: xt [512,2048] (host pre-transposed) streamed in quarters;
            QT/KT = W^T x^T per head-pair [128, n] (+bias per partition);
            V per j-tile [128, 260] bf16 with a ones column per head.
  rounds (icb 0..1 i-chunks of 1024) x (head pair p 0..1), jt 0..15:
    S^T_h [128j, 1024i] = K_h Q_h^T  (two heads row-packed via tile_position)
    P^T_h = exp(S^T_h / sqrt(512))   (ScalarE -> bf16 SBUF)
    O[i_sub, 65] += P^T_slice.T V_ext  (lhsT = P^T 128x128 slice, rhs = V_ext
       [128, 65]; col 64 accumulates the softmax row-sum -> per-partition!)
  normalize: rcp = 1/O[:,64] (DVE [128,1]); O_norm = O[:, :64] * rcp
    (tensor_scalar, per-partition broadcast) -> staged [128 i, 128 d] bf16.
  O^T via identity-matmul transpose (PE) + DVE copy -> ot[kt][128 d, n] bf16;
  z^T[f,i] = Wo^T O^T (bf16) + bias, DMA out per 512-chunk.

Scheduling: ACT must never stall. S(jt+1) is emitted before PV(jt); all other
work (projections, transposes of the previous round, z chunks) lives in a
background FIFO popped once per jt AFTER S(jt+1,h1), so a stalled item can
only delay work that has >1 jt of slack. Background items chain through the
single spare PSUM bank (pz); items are ordered so each completes before its
consumer's deadline.

Output zt [512, 2048] = z^T; host transposes back, sums partials, adds bo.
"""

import sys
import os

sys.path.insert(0, "/opt/trn_rl_repo")

import numpy as np

B, N, F = 4, 2048, 512
H, D = 8, 64
P = 128
DH = 256   # features per core (4 heads)
NPAIR = 2  # head pairs per core
KT = F // P          # 4 k-tiles over input features
ICB = 1024           # i-chunk per round
NICB = N // ICB      # 2
NT = N // P          # 16 j-tiles
NSUB = ICB // P      # 8 i-subtiles per chunk
SCALE = 1.0 / float(np.float32(F) ** 0.5)

_cache = {}


def _bf_np():
    import ml_dtypes

    return np.dtype(ml_dtypes.bfloat16)


def build():
    """Build + bass-compile the per-core program."""
    import concourse.tile as tile
    from concourse import bacc, mybir
    from contextlib import ExitStack

    f32 = mybir.dt.float32
    f32r = mybir.dt.float32r
    bf = mybir.dt.bfloat16
    AF = mybir.ActivationFunctionType

    n = N
    nc = bacc.Bacc("TRN2", target_bir_lowering=False, debug=False)

    xt_d = nc.dram_tensor("xt", [F, n], f32r, kind="ExternalInput").ap()
    wq_d = nc.dram_tensor("wq", [F, DH], f32r, kind="ExternalInput").ap()
    wk_d = nc.dram_tensor("wk", [F, DH], f32r, kind="ExternalInput").ap()
    wv_d = nc.dram_tensor("wv", [F, DH], f32r, kind="ExternalInput").ap()
    wo_d = nc.dram_tensor("wob", [DH, F], bf, kind="ExternalInput").ap()
    bq_d = nc.dram_tensor("bq2", [P, NPAIR], f32, kind="ExternalInput").ap()
    bk_d = nc.dram_tensor("bk2", [P, NPAIR], f32, kind="ExternalInput").ap()
    zb_d = nc.dram_tensor("zb4", [P, F // P], f32, kind="ExternalInput").ap()
    id_d = nc.dram_tensor("ident", [P, P], bf, kind="ExternalInput").ap()
    zt_d = nc.dram_tensor("zt", [F, n], f32, kind="ExternalOutput").ap()

    with tile.TileContext(nc) as tc, ExitStack() as ctx:
        const = ctx.enter_context(tc.tile_pool(name="const", bufs=1))
        pt_pool = ctx.enter_context(tc.tile_pool(name="pt", bufs=6))
        rc_pool = ctx.enter_context(tc.tile_pool(name="rc", bufs=8))
        on_pool = ctx.enter_context(tc.tile_pool(name="on", bufs=16))
        zs_pool = ctx.enter_context(tc.tile_pool(name="zs", bufs=4))
        ps_s = ctx.enter_context(tc.tile_pool(name="ps_s", bufs=2, space="PSUM"))
        po_pool = ctx.enter_context(tc.tile_pool(name="po", bufs=3, space="PSUM"))
        pz_pool = ctx.enter_context(tc.tile_pool(name="pz", bufs=1, space="PSUM"))

        # ---- DMA loads: first-needed first; xt in quarters so the first
        # K/Q projection chunks can start as early as possible ---------------
        xt = [const.tile([P, n], f32r, tag=f"xt{k}", name=f"xt{k}") for k in range(KT)]
        wk = [const.tile([P, DH], f32r, tag=f"wk{k}", name=f"wk{k}") for k in range(KT)]
        wq = [const.tile([P, DH], f32r, tag=f"wq{k}", name=f"wq{k}") for k in range(KT)]
        wv = [const.tile([P, DH], f32r, tag=f"wv{k}", name=f"wv{k}") for k in range(KT)]
        for k in range(KT):
            nc.sync.dma_start(xt[k][:, 0:512], xt_d[k * P:(k + 1) * P, 0:512])
            nc.sync.dma_start(wk[k][:], wk_d[k * P:(k + 1) * P, :])
        for k in range(KT):
            nc.sync.dma_start(wq[k][:], wq_d[k * P:(k + 1) * P, :])
        for k in range(KT):
            nc.sync.dma_start(xt[k][:, 512:1024], xt_d[k * P:(k + 1) * P, 512:1024])
        bq_sb = const.tile([P, NPAIR], f32, tag="bq", name="bq_sb")
        bk_sb = const.tile([P, NPAIR], f32, tag="bk", name="bk_sb")
        zb_sb = const.tile([P, F // P], f32, tag="zb", name="zb_sb")
        nc.sync.dma_start(bk_sb[:], bk_d[:])
        nc.sync.dma_start(bq_sb[:], bq_d[:])
        for k in range(KT):
            nc.sync.dma_start(wv[k][:], wv_d[k * P:(k + 1) * P, :])
        for k in range(KT):
            nc.sync.dma_start(xt[k][:, 1024:n], xt_d[k * P:(k + 1) * P, 1024:n])
        nc.sync.dma_start(zb_sb[:], zb_d[:])
        wo = [const.tile([P, F], bf, tag=f"wo{k}", name=f"wo{k}") for k in range(DH // P)]
        for k in range(DH // P):
            nc.sync.dma_start(wo[k][:], wo_d[k * P:(k + 1) * P, :])
        ident = const.tile([P, P], bf, tag="ident", name="ident")
        nc.sync.dma_start(ident[:], id_d[:])

        # warm the exp table set on ScalarE while DMAs stream in
        warm = const.tile([1, 1], f32, tag="warm", name="warm")
        nc.vector.memset(warm[:], 0.0)
        nc.scalar.activation(warm[:], warm[:], AF.Exp)

        # persistent activations
        qt = [const.tile([P, n], f32r, tag=f"qt{p}", name=f"qt{p}") for p in range(NPAIR)]
        kt_sb = [const.tile([P, n], f32r, tag=f"kt{p}", name=f"ktsb{p}") for p in range(NPAIR)]
        # V per j-tile: [128, 260] bf16, head hl at cols [65*hl, 65*hl+64),
        # ones at col 65*hl+64 (accumulates softmax row-sums in PV).
        v_sb = [const.tile([P, 4 * (D + 1)], bf, tag=f"v{j}", name=f"v{j}")
                for j in range(NT)]
        for j in range(NT):
            nc.gpsimd.memset(v_sb[j][:], 1.0)
        # O^T staging for the z projection: [128 d, n] bf16 per k-tile (pair)
        ot = [const.tile([P, n], bf, tag=f"ot{p}", name=f"ot{p}") for p in range(NPAIR)]

        def v4(ap):
            return ap.rearrange("p (h c) -> p h c", h=4)

        # ---- projections -----------------------------------------------------
        def proj_qk_half(p, w_t, b_sb, dst, ib, half, pool):
            """One 512-wide half of a Q/K projection chunk."""
            ps = pool.tile([P, 512], f32, tag="pz", name="pjh") if pool is pz_pool \
                else pool.tile([P, ICB], f32, tag="st", name="pjs")
            psl = ps[:, 0:512]
            isl = slice(ib * ICB + half * 512, ib * ICB + (half + 1) * 512)
            for k in range(KT):
                nc.tensor.matmul(
                    psl,
                    w_t[k][:, p * P:(p + 1) * P],
                    xt[k][:, isl],
                    start=(k == 0),
                    stop=(k == KT - 1),
                )
            nc.vector.tensor_scalar_add(dst[p][:, isl], psl, b_sb[:, p:p + 1])

        def proj_v_pair(j0, pool):
            """V for j-tiles j0, j0+1 sharing one pz bank (two 256-col slices)."""
            ps = pool.tile([P, 512], f32, tag="pz", name="pvh") if pool is pz_pool \
                else pool.tile([P, ICB], f32, tag="st", name="pvs")
            for m, j in enumerate((j0, j0 + 1)):
                psl = ps[:, m * DH:(m + 1) * DH]
                for k in range(KT):
                    # one start=True per pz bank; the second slice's first
                    # matmul writes pending-zero bytes (zeroed on write)
                    nc.tensor.matmul(
                        psl,
                        xt[k][:, j * P:(j + 1) * P],
                        wv[k][:],
                        start=(k == 0 and m == 0),
                        stop=(k == KT - 1 and m == 1),
                        skip_group_check=True,
                    )
            for m, j in enumerate((j0, j0 + 1)):
                psl = ps[:, m * DH:(m + 1) * DH]
                nc.vector.tensor_copy(v4(v_sb[j][:])[:, :, 0:D], v4(psl))

        def z_chunk(icb, ft, ch, pool):
            """z^T[ft*128:(ft+1)*128, 512-chunk ch of icb]."""
            if pool is pz_pool:
                zp = pool.tile([P, 512], f32, tag="pz", name="zp")
                zpl = zp[:, 0:512]
            else:
                zp = pool.tile([P, ICB], f32, tag="st", name="zps")
                zpl = zp[:, 0:512]
            isl = slice(icb * ICB + ch * 512, icb * ICB + (ch + 1) * 512)
            for k in range(DH // P):
                nc.tensor.matmul(
                    zpl,
                    wo[k][:, ft * P:(ft + 1) * P],
                    ot[k][:, isl],
                    start=(k == 0),
                    stop=(k == DH // P - 1),
                )
            zsb = zs_pool.tile([P, 512], f32, tag="zt", name="zsb")
            nc.vector.tensor_scalar_add(zsb[:], zpl, zb_sb[:, ft:ft + 1])
            nc.sync.dma_start(zt_d[ft * P:(ft + 1) * P, isl], zsb[:])

        def transp_group(p, icb, onst, grp):
            """Transpose 4 O_norm [128,64] blocks -> ot via one pz bank."""
            tp = pz_pool.tile([P, 512], f32, tag="pz", name="tp")
            for i, (h, sub) in enumerate(grp):
                nc.tensor.matmul(
                    tp[0:D, i * P:(i + 1) * P],
                    onst[sub][:, h * D:(h + 1) * D],
                    ident[:],
                    start=(i == 0),
                    stop=(i == 3),
                    skip_group_check=True,
                )
            for i, (h, sub) in enumerate(grp):
                dst = ot[p][h * D:(h + 1) * D,
                            icb * ICB + sub * P: icb * ICB + (sub + 1) * P]
                nc.vector.tensor_copy(dst, tp[0:D, i * P:(i + 1) * P])

        # prologue projections (ps_s free before the rounds; pz for V)
        for half in range(2):
            proj_qk_half(0, wk, bk_sb, kt_sb, 0, half, ps_s)
            proj_qk_half(0, wq, bq_sb, qt, 0, half, ps_s)
        proj_v_pair(0, pz_pool)

        # ---- attention rounds ------------------------------------------------
        def ob_slice(ob, h, sub):
            """PSUM accumulator slice [128, 65] for (head h, i-subtile sub)."""
            if sub < 7:
                t = ob[h]
                c0 = sub * (D + 1)
            else:
                t = ob[2]
                c0 = h * (D + 1)
            return t[:, c0:c0 + D + 1]

        def emit_S(p, icb, jt, h):
            sps = ps_s.tile([P, ICB], f32, tag="st", name="sps")
            hp = slice(D * h, D * (h + 1))
            for i5 in range(2):
                isl = slice(icb * ICB + i5 * 512, icb * ICB + (i5 + 1) * 512)
                nc.tensor.matmul(
                    sps[:, i5 * 512:(i5 + 1) * 512],
                    kt_sb[p][hp, jt * P:(jt + 1) * P],
                    qt[p][hp, isl],
                    start=True,
                    stop=True,
                    tile_position=(D * h, 0),
                )
            return sps

        rounds = [(0, 0), (0, 1), (1, 0), (1, 1)]
        bg = []          # background FIFO: fns emitting pz-chained work
        onst_by_round = {}

        for r, (icb, p) in enumerate(rounds):
            # load this round's background work (deadline-ordered)
            if r == 0:
                bg += [
                    lambda: proj_v_pair(2, pz_pool),
                    lambda: proj_v_pair(4, pz_pool),
                    lambda: proj_v_pair(6, pz_pool),
                    lambda: proj_v_pair(8, pz_pool),
                    lambda: proj_qk_half(0, wk, bk_sb, kt_sb, 1, 0, pz_pool),
                    lambda: proj_qk_half(0, wk, bk_sb, kt_sb, 1, 1, pz_pool),
                    lambda: proj_v_pair(10, pz_pool),
                    lambda: proj_v_pair(12, pz_pool),
                    lambda: proj_qk_half(1, wk, bk_sb, kt_sb, 0, 0, pz_pool),
                    lambda: proj_qk_half(1, wq, bq_sb, qt, 0, 0, pz_pool),
                    lambda: proj_v_pair(14, pz_pool),
                    lambda: proj_qk_half(1, wk, bk_sb, kt_sb, 0, 1, pz_pool),
                    lambda: proj_qk_half(1, wq, bq_sb, qt, 0, 1, pz_pool),
                ]
            elif r == 1:
                po_, oo_ = onst_by_round[0]
                items = [(h, s) for h in range(2) for s in range(NSUB)]
                bg += [
                    lambda: proj_qk_half(1, wk, bk_sb, kt_sb, 1, 0, pz_pool),
                    lambda: proj_qk_half(1, wk, bk_sb, kt_sb, 1, 1, pz_pool),
                ]
                bg += [lambda g=g, po2=po_, oo2=oo_: transp_group(
                        po2[0], po2[1], oo2, items[g * 4:(g + 1) * 4])
                       for g in range(4)]
                bg += [
                    lambda: proj_qk_half(0, wq, bq_sb, qt, 1, 0, pz_pool),
                    lambda: proj_qk_half(0, wq, bq_sb, qt, 1, 1, pz_pool),
                    lambda: proj_qk_half(1, wq, bq_sb, qt, 1, 0, pz_pool),
                    lambda: proj_qk_half(1, wq, bq_sb, qt, 1, 1, pz_pool),
                ]
            elif r == 2:
                po_, oo_ = onst_by_round[1]
                items = [(h, s) for h in range(2) for s in range(NSUB)]
                bg += [lambda g=g, po2=po_, oo2=oo_: transp_group(
                        po2[0], po2[1], oo2, items[g * 4:(g + 1) * 4])
                       for g in range(4)]
                bg += [lambda ft=ft, ch=ch: z_chunk(0, ft, ch, pz_pool)
                       for ft in range(F // P) for ch in range(2)]
            elif r == 3:
                po_, oo_ = onst_by_round[2]
                items = [(h, s) for h in range(2) for s in range(NSUB)]
                bg += [lambda g=g, po2=po_, oo2=oo_: transp_group(
                        po2[0], po2[1], oo2, items[g * 4:(g + 1) * 4])
                       for g in range(4)]

            ob = [po_pool.tile([P, 512], f32, tag="po", name=f"ob{i}")
                  for i in range(3)]
            sps_tiles = {0: emit_S(p, icb, 0, 0), 1: emit_S(p, icb, 0, 1)}
            for jt in range(NT):
                nxt = {}
                if jt + 1 < NT:
                    nxt[0] = emit_S(p, icb, jt + 1, 0)
                    # slot 1: only pop when backlog exceeds remaining jts
                    if bg and len(bg) > (NT - jt):
                        bg.pop(0)()
                    nxt[1] = emit_S(p, icb, jt + 1, 1)
                if bg:
                    bg.pop(0)()
                for h in range(2):
                    hl = 2 * p + h
                    ptile = pt_pool.tile([P, ICB], bf, tag="pt", name="pt")
                    nc.scalar.activation(ptile[:], sps_tiles[h][:], AF.Exp, scale=SCALE)
                    for sub in range(NSUB):
                        # start=True zeroes the whole 2KB psum bank row, so
                        # only the first matmul into each ob bank may set it
                        first_in_bank = (sub == 0) or (sub == 7 and h == 0)
                        nc.tensor.matmul(
                            ob_slice(ob, h, sub),
                            ptile[:, sub * P:(sub + 1) * P],
                            v_sb[jt][:, hl * (D + 1):(hl + 1) * (D + 1)],
                            start=(jt == 0 and first_in_bank),
                            stop=(jt == NT - 1),
                            skip_group_check=True,
                        )
                sps_tiles = nxt

            # ---- normalize (frees the ob banks for the next round) ----------
            onst = [on_pool.tile([P, P], bf, tag="on", name=f"on{s}")
                    for s in range(NSUB)]
            for h in range(2):
                for sub in range(NSUB):
                    sl = ob_slice(ob, h, sub)
                    rc = rc_pool.tile([P, 1], f32, tag="rc", name="rc")
                    nc.vector.reciprocal(rc[:], sl[:, D:D + 1])
                    nc.vector.tensor_scalar_mul(
                        onst[sub][:, h * D:(h + 1) * D], sl[:, 0:D], rc[:]
                    )
            onst_by_round[r] = ((p, icb), onst)

        # ---- tail: last round's transposes via the freed wide ps_s tiles,
        # then icb=1 z chunks rotating through pz + both ps_s bufs ------------
        _, onst3 = onst_by_round[3]
        items = [(h, s) for h in range(2) for s in range(NSUB)]
        for half in range(2):
            tp = ps_s.tile([P, ICB], f32, tag="st", name="tptail")
            grp8 = items[half * 8:(half + 1) * 8]
            for i, (h, sub) in enumerate(grp8):
                nc.tensor.matmul(
                    tp[0:D, i * P:(i + 1) * P],
                    onst3[sub][:, h * D:(h + 1) * D],
                    ident[:],
                    start=(i % 4 == 0),
                    stop=(i % 4 == 3),
                    skip_group_check=True,
                )
            for i, (h, sub) in enumerate(grp8):
                dst = ot[1][h * D:(h + 1) * D,
                            ICB + sub * P: ICB + (sub + 1) * P]
                nc.vector.tensor_copy(dst, tp[0:D, i * P:(i + 1) * P])
        for idx, (ft, ch) in enumerate((ft, ch) for ft in range(F // P)
                                       for ch in range(2)):
            z_chunk(1, ft, ch, pz_pool if idx % 3 == 0 else ps_s)

    nc.compile()
    return nc


def _get_nc():
    if "nc" not in _cache:
        _cache["nc"] = build()
    return _cache["nc"]


def make_in_maps(x, Wq, bq, Wk, bk, Wv, bv, Wo, bo):
    """Host-side sharding: per-core input dict for core c = 2*b + hg."""
    bfnp = _bf_np()
    in_maps = []
    for c in range(8):
        b, hg = divmod(c, 2)
        cs = slice(hg * DH, (hg + 1) * DH)
        wo_s = np.ascontiguousarray(Wo[cs, :])
        zb = np.asarray(bv[cs] @ wo_s, dtype=np.float32)
        in_maps.append({
            "xt": np.ascontiguousarray(np.asarray(x[b]).T),
            "wq": np.ascontiguousarray(Wq[:, cs]),
            "wk": np.ascontiguousarray(Wk[:, cs]),
            "wv": np.ascontiguousarray(Wv[:, cs]),
            "wob": np.ascontiguousarray(wo_s.astype(bfnp)),
            "bq2": np.ascontiguousarray(np.asarray(bq[cs]).reshape(NPAIR, P).T),
            "bk2": np.ascontiguousarray(np.asarray(bk[cs]).reshape(NPAIR, P).T),
            "zb4": np.ascontiguousarray(zb.reshape(F // P, P).T),
            "ident": np.eye(P, dtype=bfnp),
        })
    return in_maps


def kernel(x, Wq, bq, Wk, bk, Wv, bv, Wo, bo):
    from concourse.bass_utils import run_bass_kernel_spmd

    x = np.asarray(x, dtype=np.float32)
    args = [np.asarray(a, dtype=np.float32) for a in (Wq, bq, Wk, bk, Wv, bv, Wo, bo)]
    nc = _get_nc()
    in_maps = make_in_maps(x, *args)
    res = run_bass_kernel_spmd(nc, in_maps, list(range(8)))
    bo = args[-1]
    out = np.empty((B, N, F), dtype=np.float32)
    for b in range(B):
        zt0 = res.results[2 * b]["zt"]
        zt1 = res.results[2 * b + 1]["zt"]
        out[b] = (zt0 + zt1).T + bo
    return out


# revision 15
# speedup vs baseline: 1.4880x; 1.0035x over previous
"""Multi-head self-attention TRN2 Bass kernel.

Problem: x[4,2048,512], 8 heads of d=64, scale 1/sqrt(512) (full feature dim).

Sharding: 8 cores = (batch b in 0..3) x (head-group hg in 0..1). Each core
handles one batch element and 4 heads (256 of the 512 features), computing a
partial output projection z_partial = attn_heads @ Wo[hg rows].  The host
sums the two partials per batch and adds bo.

Per-core dataflow (ACT exp is the bottleneck; PE work minimized):
  prologue: xt [512,2048] (host pre-transposed) streamed in quarters;
            QT/KT = W^T x^T per head-pair [128, n] (+bias per partition);
            V per j-tile [128, 260] bf16 with a ones column per head.
  rounds (icb 0..1 i-chunks of 1024) x (head pair p 0..1), jt 0..15:
    S^T_h [128j, 1024i] = K_h Q_h^T  (two heads row-packed via tile_position)
    P^T_h = exp(S^T_h / sqrt(512))   (ScalarE -> bf16 SBUF)
    O[i_sub, 65] += P^T_slice.T V_ext  (lhsT = P^T 128x128 slice, rhs = V_ext
       [128, 65]; col 64 accumulates the softmax row-sum -> per-partition!)
  normalize: rcp = 1/O[:,64] (DVE [128,1]); O_norm = O[:, :64] * rcp
    (tensor_scalar, per-partition broadcast) -> staged [128 i, 128 d] bf16.
  O^T via identity-matmul transpose (PE) + DVE copy -> ot[kt][128 d, n] bf16;
  z^T[f,i] = Wo^T O^T (bf16) + bias, DMA out per 512-chunk.

Scheduling: ACT must never stall. S(jt+1) is emitted before PV(jt); all other
work (projections, transposes of the previous round, z chunks) lives in a
background FIFO popped once per jt AFTER S(jt+1,h1), so a stalled item can
only delay work that has >1 jt of slack. Background items chain through the
single spare PSUM bank (pz); items are ordered so each completes before its
consumer's deadline.

Output zt [512, 2048] = z^T; host transposes back, sums partials, adds bo.
"""

import sys
import os

sys.path.insert(0, "/opt/trn_rl_repo")

import numpy as np

B, N, F = 4, 2048, 512
H, D = 8, 64
P = 128
DH = 256   # features per core (4 heads)
NPAIR = 2  # head pairs per core
KT = F // P          # 4 k-tiles over input features
ICB = 1024           # i-chunk per round
NICB = N // ICB      # 2
NT = N // P          # 16 j-tiles
NSUB = ICB // P      # 8 i-subtiles per chunk
SCALE = 1.0 / float(np.float32(F) ** 0.5)

_cache = {}


def _bf_np():
    import ml_dtypes

    return np.dtype(ml_dtypes.bfloat16)


def build():
    """Build + bass-compile the per-core program."""
    import concourse.tile as tile
    from concourse import bacc, mybir
    from contextlib import ExitStack

    f32 = mybir.dt.float32
    f32r = mybir.dt.float32r
    bf = mybir.dt.bfloat16
    AF = mybir.ActivationFunctionType

    n = N
    nc = bacc.Bacc("TRN2", target_bir_lowering=False, debug=False)

    xt_d = nc.dram_tensor("xt", [F, n], f32r, kind="ExternalInput").ap()
    wq_d = nc.dram_tensor("wq", [F, DH], f32r, kind="ExternalInput").ap()
    wk_d = nc.dram_tensor("wk", [F, DH], f32r, kind="ExternalInput").ap()
    wv_d = nc.dram_tensor("wv", [F, DH], f32r, kind="ExternalInput").ap()
    wo_d = nc.dram_tensor("wob", [DH, F], bf, kind="ExternalInput").ap()
    bq_d = nc.dram_tensor("bq2", [P, NPAIR], f32, kind="ExternalInput").ap()
    bk_d = nc.dram_tensor("bk2", [P, NPAIR], f32, kind="ExternalInput").ap()
    zb_d = nc.dram_tensor("zb4", [P, F // P], f32, kind="ExternalInput").ap()
    id_d = nc.dram_tensor("ident", [P, P], bf, kind="ExternalInput").ap()
    zt_d = nc.dram_tensor("zt", [F, n], f32, kind="ExternalOutput").ap()

    with tile.TileContext(nc) as tc, ExitStack() as ctx:
        const = ctx.enter_context(tc.tile_pool(name="const", bufs=1))
        pt_pool = ctx.enter_context(tc.tile_pool(name="pt", bufs=6))
        rc_pool = ctx.enter_context(tc.tile_pool(name="rc", bufs=8))
        on_pool = ctx.enter_context(tc.tile_pool(name="on", bufs=16))
        zs_pool = ctx.enter_context(tc.tile_pool(name="zs", bufs=4))
        ps_s = ctx.enter_context(tc.tile_pool(name="ps_s", bufs=2, space="PSUM"))
        po_pool = ctx.enter_context(tc.tile_pool(name="po", bufs=3, space="PSUM"))
        pz_pool = ctx.enter_context(tc.tile_pool(name="pz", bufs=1, space="PSUM"))

        # ---- DMA loads: first-needed first; xt in quarters so the first
        # K/Q projection chunks can start as early as possible ---------------
        xt = [const.tile([P, n], f32r, tag=f"xt{k}", name=f"xt{k}") for k in range(KT)]
        wk = [const.tile([P, DH], f32r, tag=f"wk{k}", name=f"wk{k}") for k in range(KT)]
        wq = [const.tile([P, DH], f32r, tag=f"wq{k}", name=f"wq{k}") for k in range(KT)]
        wv = [const.tile([P, DH], f32r, tag=f"wv{k}", name=f"wv{k}") for k in range(KT)]
        bq_sb = const.tile([P, NPAIR], f32, tag="bq", name="bq_sb")
        bk_sb = const.tile([P, NPAIR], f32, tag="bk", name="bk_sb")
        zb_sb = const.tile([P, F // P], f32, tag="zb", name="zb_sb")
        nc.sync.dma_start(bk_sb[:], bk_d[:])
        nc.sync.dma_start(bq_sb[:], bq_d[:])
        for k in range(KT):
            nc.sync.dma_start(xt[k][:, 0:512], xt_d[k * P:(k + 1) * P, 0:512])
            nc.sync.dma_start(wk[k][:], wk_d[k * P:(k + 1) * P, :])
        for k in range(KT):
            nc.sync.dma_start(wq[k][:], wq_d[k * P:(k + 1) * P, :])
        for k in range(KT):
            nc.sync.dma_start(xt[k][:, 512:1024], xt_d[k * P:(k + 1) * P, 512:1024])
        for k in range(KT):
            nc.sync.dma_start(wv[k][:], wv_d[k * P:(k + 1) * P, :])
        for k in range(KT):
            nc.sync.dma_start(xt[k][:, 1024:n], xt_d[k * P:(k + 1) * P, 1024:n])
        nc.sync.dma_start(zb_sb[:], zb_d[:])
        wo = [const.tile([P, F], bf, tag=f"wo{k}", name=f"wo{k}") for k in range(DH // P)]
        for k in range(DH // P):
            nc.sync.dma_start(wo[k][:], wo_d[k * P:(k + 1) * P, :])
        ident = const.tile([P, P], bf, tag="ident", name="ident")
        nc.sync.dma_start(ident[:], id_d[:])

        # warm the exp table set on ScalarE while DMAs stream in
        warm = const.tile([1, 1], f32, tag="warm", name="warm")
        nc.vector.memset(warm[:], 0.0)
        nc.scalar.activation(warm[:], warm[:], AF.Exp)

        # persistent activations
        qt = [const.tile([P, n], f32r, tag=f"qt{p}", name=f"qt{p}") for p in range(NPAIR)]
        kt_sb = [const.tile([P, n], f32r, tag=f"kt{p}", name=f"ktsb{p}") for p in range(NPAIR)]
        # V per j-tile: [128, 260] bf16, head hl at cols [65*hl, 65*hl+64),
        # ones at col 65*hl+64 (accumulates softmax row-sums in PV).
        v_sb = [const.tile([P, 4 * (D + 1)], bf, tag=f"v{j}", name=f"v{j}")
                for j in range(NT)]
        for j in range(NT):
            nc.gpsimd.memset(v_sb[j][:], 1.0)
        # O^T staging for the z projection: [128 d, n] bf16 per k-tile (pair)
        ot = [const.tile([P, n], bf, tag=f"ot{p}", name=f"ot{p}") for p in range(NPAIR)]

        def v4(ap):
            return ap.rearrange("p (h c) -> p h c", h=4)

        # ---- projections -----------------------------------------------------
        def proj_qk_half(p, w_t, b_sb, dst, ib, half, pool):
            """One 512-wide half of a Q/K projection chunk."""
            ps = pool.tile([P, 512], f32, tag="pz", name="pjh") if pool is pz_pool \
                else pool.tile([P, ICB], f32, tag="st", name="pjs")
            psl = ps[:, 0:512]
            isl = slice(ib * ICB + half * 512, ib * ICB + (half + 1) * 512)
            for k in range(KT):
                nc.tensor.matmul(
                    psl,
                    w_t[k][:, p * P:(p + 1) * P],
                    xt[k][:, isl],
                    start=(k == 0),
                    stop=(k == KT - 1),
                )
            nc.vector.tensor_scalar_add(dst[p][:, isl], psl, b_sb[:, p:p + 1])

        def proj_v_pair(j0, pool):
            """V for j-tiles j0, j0+1 sharing one pz bank (two 256-col slices)."""
            ps = pool.tile([P, 512], f32, tag="pz", name="pvh") if pool is pz_pool \
                else pool.tile([P, ICB], f32, tag="st", name="pvs")
            for m, j in enumerate((j0, j0 + 1)):
                psl = ps[:, m * DH:(m + 1) * DH]
                for k in range(KT):
                    # one start=True per pz bank; the second slice's first
                    # matmul writes pending-zero bytes (zeroed on write)
                    nc.tensor.matmul(
                        psl,
                        xt[k][:, j * P:(j + 1) * P],
                        wv[k][:],
                        start=(k == 0 and m == 0),
                        stop=(k == KT - 1 and m == 1),
                        skip_group_check=True,
                    )
            for m, j in enumerate((j0, j0 + 1)):
                psl = ps[:, m * DH:(m + 1) * DH]
                nc.vector.tensor_copy(v4(v_sb[j][:])[:, :, 0:D], v4(psl))

        def z_chunk(icb, ft, ch, pool):
            """z^T[ft*128:(ft+1)*128, 512-chunk ch of icb]."""
            if pool is pz_pool:
                zp = pool.tile([P, 512], f32, tag="pz", name="zp")
                zpl = zp[:, 0:512]
            else:
                zp = pool.tile([P, ICB], f32, tag="st", name="zps")
                zpl = zp[:, 0:512]
            isl = slice(icb * ICB + ch * 512, icb * ICB + (ch + 1) * 512)
            for k in range(DH // P):
                nc.tensor.matmul(
                    zpl,
                    wo[k][:, ft * P:(ft + 1) * P],
                    ot[k][:, isl],
                    start=(k == 0),
                    stop=(k == DH // P - 1),
                )
            zsb = zs_pool.tile([P, 512], f32, tag="zt", name="zsb")
            nc.vector.tensor_scalar_add(zsb[:], zpl, zb_sb[:, ft:ft + 1])
            nc.sync.dma_start(zt_d[ft * P:(ft + 1) * P, isl], zsb[:])

        def transp_group(p, icb, onst, grp):
            """Transpose 4 O_norm [128,64] blocks -> ot via one pz bank."""
            tp = pz_pool.tile([P, 512], f32, tag="pz", name="tp")
            for i, (h, sub) in enumerate(grp):
                nc.tensor.matmul(
                    tp[0:D, i * P:(i + 1) * P],
                    onst[sub][:, h * D:(h + 1) * D],
                    ident[:],
                    start=(i == 0),
                    stop=(i == 3),
                    skip_group_check=True,
                )
            for i, (h, sub) in enumerate(grp):
                dst = ot[p][h * D:(h + 1) * D,
                            icb * ICB + sub * P: icb * ICB + (sub + 1) * P]
                nc.vector.tensor_copy(dst, tp[0:D, i * P:(i + 1) * P])

        # prologue projections (ps_s free before the rounds; pz for V)
        for half in range(2):
            proj_qk_half(0, wk, bk_sb, kt_sb, 0, half, ps_s)
            proj_qk_half(0, wq, bq_sb, qt, 0, half, ps_s)
        proj_v_pair(0, pz_pool)

        # ---- attention rounds ------------------------------------------------
        def ob_slice(ob, h, sub):
            """PSUM accumulator slice [128, 65] for (head h, i-subtile sub)."""
            if sub < 7:
                t = ob[h]
                c0 = sub * (D + 1)
            else:
                t = ob[2]
                c0 = h * (D + 1)
            return t[:, c0:c0 + D + 1]

        def emit_S(p, icb, jt, h):
            sps = ps_s.tile([P, ICB], f32, tag="st", name="sps")
            hp = slice(D * h, D * (h + 1))
            for i5 in range(2):
                isl = slice(icb * ICB + i5 * 512, icb * ICB + (i5 + 1) * 512)
                nc.tensor.matmul(
                    sps[:, i5 * 512:(i5 + 1) * 512],
                    kt_sb[p][hp, jt * P:(jt + 1) * P],
                    qt[p][hp, isl],
                    start=True,
                    stop=True,
                    tile_position=(D * h, 0),
                )
            return sps

        rounds = [(0, 0), (0, 1), (1, 0), (1, 1)]
        bg = []          # background FIFO: fns emitting pz-chained work
        onst_by_round = {}

        for r, (icb, p) in enumerate(rounds):
            # load this round's background work (deadline-ordered)
            if r == 0:
                bg += [
                    lambda: proj_v_pair(2, pz_pool),
                    lambda: proj_v_pair(4, pz_pool),
                    lambda: proj_v_pair(6, pz_pool),
                    lambda: proj_v_pair(8, pz_pool),
                    lambda: proj_qk_half(0, wk, bk_sb, kt_sb, 1, 0, pz_pool),
                    lambda: proj_qk_half(0, wk, bk_sb, kt_sb, 1, 1, pz_pool),
                    lambda: proj_v_pair(10, pz_pool),
                    lambda: proj_v_pair(12, pz_pool),
                    lambda: proj_qk_half(1, wk, bk_sb, kt_sb, 0, 0, pz_pool),
                    lambda: proj_qk_half(1, wq, bq_sb, qt, 0, 0, pz_pool),
                    lambda: proj_v_pair(14, pz_pool),
                    lambda: proj_qk_half(1, wk, bk_sb, kt_sb, 0, 1, pz_pool),
                    lambda: proj_qk_half(1, wq, bq_sb, qt, 0, 1, pz_pool),
                ]
            elif r == 1:
                po_, oo_ = onst_by_round[0]
                items = [(h, s) for h in range(2) for s in range(NSUB)]
                bg += [
                    lambda: proj_qk_half(1, wk, bk_sb, kt_sb, 1, 0, pz_pool),
                    lambda: proj_qk_half(1, wk, bk_sb, kt_sb, 1, 1, pz_pool),
                ]
                bg += [lambda g=g, po2=po_, oo2=oo_: transp_group(
                        po2[0], po2[1], oo2, items[g * 4:(g + 1) * 4])
                       for g in range(4)]
                bg += [
                    lambda: proj_qk_half(0, wq, bq_sb, qt, 1, 0, pz_pool),
                    lambda: proj_qk_half(0, wq, bq_sb, qt, 1, 1, pz_pool),
                    lambda: proj_qk_half(1, wq, bq_sb, qt, 1, 0, pz_pool),
                    lambda: proj_qk_half(1, wq, bq_sb, qt, 1, 1, pz_pool),
                ]
            elif r == 2:
                po_, oo_ = onst_by_round[1]
                items = [(h, s) for h in range(2) for s in range(NSUB)]
                bg += [lambda g=g, po2=po_, oo2=oo_: transp_group(
                        po2[0], po2[1], oo2, items[g * 4:(g + 1) * 4])
                       for g in range(4)]
                bg += [lambda ft=ft, ch=ch: z_chunk(0, ft, ch, pz_pool)
                       for ft in range(F // P) for ch in range(2)]
            elif r == 3:
                po_, oo_ = onst_by_round[2]
                items = [(h, s) for h in range(2) for s in range(NSUB)]
                bg += [lambda g=g, po2=po_, oo2=oo_: transp_group(
                        po2[0], po2[1], oo2, items[g * 4:(g + 1) * 4])
                       for g in range(4)]

            ob = [po_pool.tile([P, 512], f32, tag="po", name=f"ob{i}")
                  for i in range(3)]
            sps_tiles = {0: emit_S(p, icb, 0, 0), 1: emit_S(p, icb, 0, 1)}
            for jt in range(NT):
                nxt = {}
                if jt + 1 < NT:
                    nxt[0] = emit_S(p, icb, jt + 1, 0)
                    # slot 1: only pop when backlog exceeds remaining jts
                    if bg and len(bg) > (NT - jt):
                        bg.pop(0)()
                    nxt[1] = emit_S(p, icb, jt + 1, 1)
                if bg:
                    bg.pop(0)()
                for h in range(2):
                    hl = 2 * p + h
                    ptile = pt_pool.tile([P, ICB], bf, tag="pt", name="pt")
                    nc.scalar.activation(ptile[:], sps_tiles[h][:], AF.Exp, scale=SCALE)
                    for sub in range(NSUB):
                        # start=True zeroes the whole 2KB psum bank row, so
                        # only the first matmul into each ob bank may set it
                        first_in_bank = (sub == 0) or (sub == 7 and h == 0)
                        nc.tensor.matmul(
                            ob_slice(ob, h, sub),
                            ptile[:, sub * P:(sub + 1) * P],
                            v_sb[jt][:, hl * (D + 1):(hl + 1) * (D + 1)],
                            start=(jt == 0 and first_in_bank),
                            stop=(jt == NT - 1),
                            skip_group_check=True,
                        )
                sps_tiles = nxt

            # ---- normalize (frees the ob banks for the next round) ----------
            onst = [on_pool.tile([P, P], bf, tag="on", name=f"on{s}")
                    for s in range(NSUB)]
            for h in range(2):
                for sub in range(NSUB):
                    sl = ob_slice(ob, h, sub)
                    rc = rc_pool.tile([P, 1], f32, tag="rc", name="rc")
                    nc.vector.reciprocal(rc[:], sl[:, D:D + 1])
                    dst = onst[sub][:, h * D:(h + 1) * D]
                    if r == 3 and sub % 2 == h:
                        # tail: ACT is idle -> per-partition scale via Copy
                        nc.scalar.activation(dst, sl[:, 0:D], AF.Copy, scale=rc[:])
                    else:
                        nc.vector.tensor_scalar_mul(dst, sl[:, 0:D], rc[:])
            onst_by_round[r] = ((p, icb), onst)

        # ---- tail: last round's transposes via the freed wide ps_s tiles,
        # then icb=1 z chunks rotating through pz + both ps_s bufs ------------
        _, onst3 = onst_by_round[3]
        items = [(h, s) for h in range(2) for s in range(NSUB)]
        for half in range(2):
            tp = ps_s.tile([P, ICB], f32, tag="st", name="tptail")
            grp8 = items[half * 8:(half + 1) * 8]
            for i, (h, sub) in enumerate(grp8):
                nc.tensor.matmul(
                    tp[0:D, i * P:(i + 1) * P],
                    onst3[sub][:, h * D:(h + 1) * D],
                    ident[:],
                    start=(i % 4 == 0),
                    stop=(i % 4 == 3),
                    skip_group_check=True,
                )
            for i, (h, sub) in enumerate(grp8):
                dst = ot[1][h * D:(h + 1) * D,
                            ICB + sub * P: ICB + (sub + 1) * P]
                if i % 2 == 0:
                    nc.scalar.activation(dst, tp[0:D, i * P:(i + 1) * P], AF.Copy)
                else:
                    nc.vector.tensor_copy(dst, tp[0:D, i * P:(i + 1) * P])
        # icb=1 z: one wide psum tile per f-tile; bias-add alternates ACT/DVE
        for ft in range(F // P):
            zp = ps_s.tile([P, ICB], f32, tag="st", name="zptail")
            for ch in range(2):
                isl = slice(ICB + ch * 512, ICB + (ch + 1) * 512)
                for k in range(DH // P):
                    nc.tensor.matmul(
                        zp[:, ch * 512:(ch + 1) * 512],
                        wo[k][:, ft * P:(ft + 1) * P],
                        ot[k][:, isl],
                        start=(k == 0),
                        stop=(k == DH // P - 1),
                    )
            zsb = zs_pool.tile([P, ICB], f32, tag="ztw", name="zsbw", bufs=2)
            if ft % 2 == 0:
                nc.scalar.activation(zsb[:], zp[:], AF.Identity,
                                     bias=zb_sb[:, ft:ft + 1])
            else:
                nc.vector.tensor_scalar_add(zsb[:], zp[:], zb_sb[:, ft:ft + 1])
            nc.sync.dma_start(zt_d[ft * P:(ft + 1) * P, ICB:n], zsb[:])

    nc.compile()
    return nc


def _get_nc():
    if "nc" not in _cache:
        _cache["nc"] = build()
    return _cache["nc"]


def make_in_maps(x, Wq, bq, Wk, bk, Wv, bv, Wo, bo):
    """Host-side sharding: per-core input dict for core c = 2*b + hg."""
    bfnp = _bf_np()
    in_maps = []
    for c in range(8):
        b, hg = divmod(c, 2)
        cs = slice(hg * DH, (hg + 1) * DH)
        wo_s = np.ascontiguousarray(Wo[cs, :])
        zb = np.asarray(bv[cs] @ wo_s, dtype=np.float32)
        in_maps.append({
            "xt": np.ascontiguousarray(np.asarray(x[b]).T),
            "wq": np.ascontiguousarray(Wq[:, cs]),
            "wk": np.ascontiguousarray(Wk[:, cs]),
            "wv": np.ascontiguousarray(Wv[:, cs]),
            "wob": np.ascontiguousarray(wo_s.astype(bfnp)),
            "bq2": np.ascontiguousarray(np.asarray(bq[cs]).reshape(NPAIR, P).T),
            "bk2": np.ascontiguousarray(np.asarray(bk[cs]).reshape(NPAIR, P).T),
            "zb4": np.ascontiguousarray(zb.reshape(F // P, P).T),
            "ident": np.eye(P, dtype=bfnp),
        })
    return in_maps


def kernel(x, Wq, bq, Wk, bk, Wv, bv, Wo, bo):
    from concourse.bass_utils import run_bass_kernel_spmd

    x = np.asarray(x, dtype=np.float32)
    args = [np.asarray(a, dtype=np.float32) for a in (Wq, bq, Wk, bk, Wv, bv, Wo, bo)]
    nc = _get_nc()
    in_maps = make_in_maps(x, *args)
    res = run_bass_kernel_spmd(nc, in_maps, list(range(8)))
    bo = args[-1]
    out = np.empty((B, N, F), dtype=np.float32)
    for b in range(B):
        zt0 = res.results[2 * b]["zt"]
        zt1 = res.results[2 * b + 1]["zt"]
        out[b] = (zt0 + zt1).T + bo
    return out


# revision 17
# speedup vs baseline: 1.4975x; 1.0064x over previous
"""Multi-head self-attention TRN2 Bass kernel.

Problem: x[4,2048,512], 8 heads of d=64, scale 1/sqrt(512) (full feature dim).

Sharding: 8 cores = (batch b in 0..3) x (head-group hg in 0..1). Each core
handles one batch element and 4 heads (256 of the 512 features), computing a
partial output projection z_partial = attn_heads @ Wo[hg rows].  The host
sums the two partials per batch and adds bo.

Per-core dataflow (ACT exp is the bottleneck; PE work minimized):
  prologue: xt [512,2048] (host pre-transposed) streamed in quarters;
            QT/KT = W^T x^T per head-pair [128, n] (+bias per partition);
            V per j-tile [128, 260] bf16 with a ones column per head.
  rounds (icb 0..1 i-chunks of 1024) x (head pair p 0..1), jt 0..15:
    S^T_h [128j, 1024i] = K_h Q_h^T  (two heads row-packed via tile_position)
    P^T_h = exp(S^T_h / sqrt(512))   (ScalarE -> bf16 SBUF)
    O[i_sub, 65] += P^T_slice.T V_ext  (lhsT = P^T 128x128 slice, rhs = V_ext
       [128, 65]; col 64 accumulates the softmax row-sum -> per-partition!)
  normalize: rcp = 1/O[:,64] (DVE [128,1]); O_norm = O[:, :64] * rcp
    (tensor_scalar, per-partition broadcast) -> staged [128 i, 128 d] bf16.
  O^T via identity-matmul transpose (PE) + DVE copy -> ot[kt][128 d, n] bf16;
  z^T[f,i] = Wo^T O^T (bf16) + bias, DMA out per 512-chunk.

Scheduling: ACT must never stall. S(jt+1) is emitted before PV(jt); all other
work (projections, transposes of the previous round, z chunks) lives in a
background FIFO popped once per jt AFTER S(jt+1,h1), so a stalled item can
only delay work that has >1 jt of slack. Background items chain through the
single spare PSUM bank (pz); items are ordered so each completes before its
consumer's deadline.

Output zt [512, 2048] = z^T; host transposes back, sums partials, adds bo.
"""

import sys
import os

sys.path.insert(0, "/opt/trn_rl_repo")

import numpy as np

B, N, F = 4, 2048, 512
H, D = 8, 64
P = 128
DH = 256   # features per core (4 heads)
NPAIR = 2  # head pairs per core
KT = F // P          # 4 k-tiles over input features
ICB = 1024           # i-chunk per round
NICB = N // ICB      # 2
NT = N // P          # 16 j-tiles
NSUB = ICB // P      # 8 i-subtiles per chunk
SCALE = 1.0 / float(np.float32(F) ** 0.5)

_cache = {}


def _bf_np():
    import ml_dtypes

    return np.dtype(ml_dtypes.bfloat16)


def build():
    """Build + bass-compile the per-core program."""
    import concourse.tile as tile
    from concourse import bacc, mybir
    from contextlib import ExitStack

    f32 = mybir.dt.float32
    f32r = mybir.dt.float32r
    bf = mybir.dt.bfloat16
    AF = mybir.ActivationFunctionType

    n = N
    nc = bacc.Bacc("TRN2", target_bir_lowering=False, debug=False)

    xt_d = nc.dram_tensor("xt", [F, n], f32r, kind="ExternalInput").ap()
    wq_d = nc.dram_tensor("wq", [F, DH], f32r, kind="ExternalInput").ap()
    wk_d = nc.dram_tensor("wk", [F, DH], f32r, kind="ExternalInput").ap()
    wv_d = nc.dram_tensor("wv", [F, DH], f32r, kind="ExternalInput").ap()
    wo_d = nc.dram_tensor("wob", [DH, F], bf, kind="ExternalInput").ap()
    bq_d = nc.dram_tensor("bq2", [P, NPAIR], f32, kind="ExternalInput").ap()
    bk_d = nc.dram_tensor("bk2", [P, NPAIR], f32, kind="ExternalInput").ap()
    zb_d = nc.dram_tensor("zb4", [P, F // P], f32, kind="ExternalInput").ap()
    id_d = nc.dram_tensor("ident", [P, P], bf, kind="ExternalInput").ap()
    zt_d = nc.dram_tensor("zt", [F, n], f32, kind="ExternalOutput").ap()

    with tile.TileContext(nc) as tc, ExitStack() as ctx:
        const = ctx.enter_context(tc.tile_pool(name="const", bufs=1))
        pt_pool = ctx.enter_context(tc.tile_pool(name="pt", bufs=6))
        rc_pool = ctx.enter_context(tc.tile_pool(name="rc", bufs=8))
        on_pool = ctx.enter_context(tc.tile_pool(name="on", bufs=16))
        zs_pool = ctx.enter_context(tc.tile_pool(name="zs", bufs=4))
        ps_s = ctx.enter_context(tc.tile_pool(name="ps_s", bufs=2, space="PSUM"))
        po_pool = ctx.enter_context(tc.tile_pool(name="po", bufs=3, space="PSUM"))
        pz_pool = ctx.enter_context(tc.tile_pool(name="pz", bufs=1, space="PSUM"))

        # ---- DMA loads: first-needed first; xt in quarters so the first
        # K/Q projection chunks can start as early as possible ---------------
        xt = [const.tile([P, n], f32r, tag=f"xt{k}", name=f"xt{k}") for k in range(KT)]
        wk = [const.tile([P, DH], f32r, tag=f"wk{k}", name=f"wk{k}") for k in range(KT)]
        wq = [const.tile([P, DH], f32r, tag=f"wq{k}", name=f"wq{k}") for k in range(KT)]
        wv = [const.tile([P, DH], f32r, tag=f"wv{k}", name=f"wv{k}") for k in range(KT)]
        bq_sb = const.tile([P, NPAIR], f32, tag="bq", name="bq_sb")
        bk_sb = const.tile([P, NPAIR], f32, tag="bk", name="bk_sb")
        zb_sb = const.tile([P, F // P], f32, tag="zb", name="zb_sb")
        for k in range(KT):
            nc.sync.dma_start(xt[k][:, 0:ICB], xt_d[k * P:(k + 1) * P, 0:ICB])
            nc.sync.dma_start(wk[k][:], wk_d[k * P:(k + 1) * P, :])
        nc.sync.dma_start(bk_sb[:], bk_d[:])
        for k in range(KT):
            nc.sync.dma_start(wq[k][:], wq_d[k * P:(k + 1) * P, :])
        nc.sync.dma_start(bq_sb[:], bq_d[:])
        for k in range(KT):
            nc.sync.dma_start(wv[k][:], wv_d[k * P:(k + 1) * P, :])
        for k in range(KT):
            nc.sync.dma_start(xt[k][:, 1024:n], xt_d[k * P:(k + 1) * P, 1024:n])
        nc.sync.dma_start(zb_sb[:], zb_d[:])
        wo = [const.tile([P, F], bf, tag=f"wo{k}", name=f"wo{k}") for k in range(DH // P)]
        for k in range(DH // P):
            nc.sync.dma_start(wo[k][:], wo_d[k * P:(k + 1) * P, :])
        ident = const.tile([P, P], bf, tag="ident", name="ident")
        nc.sync.dma_start(ident[:], id_d[:])

        # warm the exp table set on ScalarE while DMAs stream in
        warm = const.tile([1, 1], f32, tag="warm", name="warm")
        nc.vector.memset(warm[:], 0.0)
        nc.scalar.activation(warm[:], warm[:], AF.Exp)

        # persistent activations
        qt = [const.tile([P, n], f32r, tag=f"qt{p}", name=f"qt{p}") for p in range(NPAIR)]
        kt_sb = [const.tile([P, n], f32r, tag=f"kt{p}", name=f"ktsb{p}") for p in range(NPAIR)]
        # V per j-tile: [128, 260] bf16, head hl at cols [65*hl, 65*hl+64),
        # ones at col 65*hl+64 (accumulates softmax row-sums in PV).
        v_sb = [const.tile([P, 4 * (D + 1)], bf, tag=f"v{j}", name=f"v{j}")
                for j in range(NT)]
        for j in range(NT):
            nc.gpsimd.memset(v_sb[j][:], 1.0)
        # O^T staging for the z projection: [128 d, n] bf16 per k-tile (pair)
        ot = [const.tile([P, n], bf, tag=f"ot{p}", name=f"ot{p}") for p in range(NPAIR)]

        def v4(ap):
            return ap.rearrange("p (h c) -> p h c", h=4)

        # ---- projections -----------------------------------------------------
        def proj_qk_half(p, w_t, b_sb, dst, ib, half, pool):
            """One 512-wide half of a Q/K projection chunk."""
            ps = pool.tile([P, 512], f32, tag="pz", name="pjh") if pool is pz_pool \
                else pool.tile([P, ICB], f32, tag="st", name="pjs")
            psl = ps[:, 0:512]
            isl = slice(ib * ICB + half * 512, ib * ICB + (half + 1) * 512)
            for k in range(KT):
                nc.tensor.matmul(
                    psl,
                    w_t[k][:, p * P:(p + 1) * P],
                    xt[k][:, isl],
                    start=(k == 0),
                    stop=(k == KT - 1),
                )
            nc.vector.tensor_scalar_add(dst[p][:, isl], psl, b_sb[:, p:p + 1])

        def proj_v_pair(j0, pool):
            """V for j-tiles j0, j0+1 sharing one pz bank (two 256-col slices)."""
            ps = pool.tile([P, 512], f32, tag="pz", name="pvh") if pool is pz_pool \
                else pool.tile([P, ICB], f32, tag="st", name="pvs")
            for m, j in enumerate((j0, j0 + 1)):
                psl = ps[:, m * DH:(m + 1) * DH]
                for k in range(KT):
                    # one start=True per pz bank; the second slice's first
                    # matmul writes pending-zero bytes (zeroed on write)
                    nc.tensor.matmul(
                        psl,
                        xt[k][:, j * P:(j + 1) * P],
                        wv[k][:],
                        start=(k == 0 and m == 0),
                        stop=(k == KT - 1 and m == 1),
                        skip_group_check=True,
                    )
            for m, j in enumerate((j0, j0 + 1)):
                psl = ps[:, m * DH:(m + 1) * DH]
                nc.vector.tensor_copy(v4(v_sb[j][:])[:, :, 0:D], v4(psl))

        def z_chunk(icb, ft, ch, pool):
            """z^T[ft*128:(ft+1)*128, 512-chunk ch of icb]."""
            if pool is pz_pool:
                zp = pool.tile([P, 512], f32, tag="pz", name="zp")
                zpl = zp[:, 0:512]
            else:
                zp = pool.tile([P, ICB], f32, tag="st", name="zps")
                zpl = zp[:, 0:512]
            isl = slice(icb * ICB + ch * 512, icb * ICB + (ch + 1) * 512)
            for k in range(DH // P):
                nc.tensor.matmul(
                    zpl,
                    wo[k][:, ft * P:(ft + 1) * P],
                    ot[k][:, isl],
                    start=(k == 0),
                    stop=(k == DH // P - 1),
                )
            zsb = zs_pool.tile([P, 512], f32, tag="zt", name="zsb")
            nc.vector.tensor_scalar_add(zsb[:], zpl, zb_sb[:, ft:ft + 1])
            nc.sync.dma_start(zt_d[ft * P:(ft + 1) * P, isl], zsb[:])

        def transp_group(p, icb, onst, grp):
            """Transpose 4 O_norm [128,64] blocks -> ot via one pz bank."""
            tp = pz_pool.tile([P, 512], f32, tag="pz", name="tp")
            for i, (h, sub) in enumerate(grp):
                nc.tensor.matmul(
                    tp[0:D, i * P:(i + 1) * P],
                    onst[sub][:, h * D:(h + 1) * D],
                    ident[:],
                    start=(i == 0),
                    stop=(i == 3),
                    skip_group_check=True,
                )
            for i, (h, sub) in enumerate(grp):
                dst = ot[p][h * D:(h + 1) * D,
                            icb * ICB + sub * P: icb * ICB + (sub + 1) * P]
                nc.vector.tensor_copy(dst, tp[0:D, i * P:(i + 1) * P])

        # prologue projections (ps_s free before the rounds; pz for V)
        for half in range(2):
            proj_qk_half(0, wk, bk_sb, kt_sb, 0, half, ps_s)
            proj_qk_half(0, wq, bq_sb, qt, 0, half, ps_s)
        proj_v_pair(0, pz_pool)

        # ---- attention rounds ------------------------------------------------
        def ob_slice(ob, h, sub):
            """PSUM accumulator slice [128, 65] for (head h, i-subtile sub)."""
            if sub < 7:
                t = ob[h]
                c0 = sub * (D + 1)
            else:
                t = ob[2]
                c0 = h * (D + 1)
            return t[:, c0:c0 + D + 1]

        def emit_S(p, icb, jt, h):
            sps = ps_s.tile([P, ICB], f32, tag="st", name="sps")
            hp = slice(D * h, D * (h + 1))
            for i5 in range(2):
                isl = slice(icb * ICB + i5 * 512, icb * ICB + (i5 + 1) * 512)
                nc.tensor.matmul(
                    sps[:, i5 * 512:(i5 + 1) * 512],
                    kt_sb[p][hp, jt * P:(jt + 1) * P],
                    qt[p][hp, isl],
                    start=True,
                    stop=True,
                    tile_position=(D * h, 0),
                )
            return sps

        rounds = [(0, 0), (0, 1), (1, 0), (1, 1)]
        bg = []          # background FIFO: fns emitting pz-chained work
        onst_by_round = {}

        for r, (icb, p) in enumerate(rounds):
            # load this round's background work (deadline-ordered)
            if r == 0:
                bg += [
                    lambda: proj_v_pair(2, pz_pool),
                    lambda: proj_v_pair(4, pz_pool),
                    lambda: proj_v_pair(6, pz_pool),
                    lambda: proj_v_pair(8, pz_pool),
                    lambda: proj_qk_half(0, wk, bk_sb, kt_sb, 1, 0, pz_pool),
                    lambda: proj_qk_half(0, wk, bk_sb, kt_sb, 1, 1, pz_pool),
                    lambda: proj_v_pair(10, pz_pool),
                    lambda: proj_v_pair(12, pz_pool),
                    lambda: proj_qk_half(1, wk, bk_sb, kt_sb, 0, 0, pz_pool),
                    lambda: proj_qk_half(1, wq, bq_sb, qt, 0, 0, pz_pool),
                    lambda: proj_v_pair(14, pz_pool),
                    lambda: proj_qk_half(1, wk, bk_sb, kt_sb, 0, 1, pz_pool),
                    lambda: proj_qk_half(1, wq, bq_sb, qt, 0, 1, pz_pool),
                ]
            elif r == 1:
                po_, oo_ = onst_by_round[0]
                items = [(h, s) for h in range(2) for s in range(NSUB)]
                bg += [
                    lambda: proj_qk_half(1, wk, bk_sb, kt_sb, 1, 0, pz_pool),
                    lambda: proj_qk_half(1, wk, bk_sb, kt_sb, 1, 1, pz_pool),
                ]
                bg += [lambda g=g, po2=po_, oo2=oo_: transp_group(
                        po2[0], po2[1], oo2, items[g * 4:(g + 1) * 4])
                       for g in range(4)]
                bg += [
                    lambda: proj_qk_half(0, wq, bq_sb, qt, 1, 0, pz_pool),
                    lambda: proj_qk_half(0, wq, bq_sb, qt, 1, 1, pz_pool),
                    lambda: proj_qk_half(1, wq, bq_sb, qt, 1, 0, pz_pool),
                    lambda: proj_qk_half(1, wq, bq_sb, qt, 1, 1, pz_pool),
                ]
            elif r == 2:
                po_, oo_ = onst_by_round[1]
                items = [(h, s) for h in range(2) for s in range(NSUB)]
                bg += [lambda g=g, po2=po_, oo2=oo_: transp_group(
                        po2[0], po2[1], oo2, items[g * 4:(g + 1) * 4])
                       for g in range(4)]
                bg += [lambda ft=ft, ch=ch: z_chunk(0, ft, ch, pz_pool)
                       for ft in range(F // P) for ch in range(2)]
            elif r == 3:
                po_, oo_ = onst_by_round[2]
                items = [(h, s) for h in range(2) for s in range(NSUB)]
                bg += [lambda g=g, po2=po_, oo2=oo_: transp_group(
                        po2[0], po2[1], oo2, items[g * 4:(g + 1) * 4])
                       for g in range(4)]

            ob = [po_pool.tile([P, 512], f32, tag="po", name=f"ob{i}")
                  for i in range(3)]
            sps_tiles = {0: emit_S(p, icb, 0, 0), 1: emit_S(p, icb, 0, 1)}
            for jt in range(NT):
                nxt = {}
                if jt + 1 < NT:
                    nxt[0] = emit_S(p, icb, jt + 1, 0)
                    # slot 1: only pop when backlog exceeds remaining jts
                    if bg and len(bg) > (NT - jt):
                        bg.pop(0)()
                    nxt[1] = emit_S(p, icb, jt + 1, 1)
                if bg:
                    bg.pop(0)()
                for h in range(2):
                    hl = 2 * p + h
                    ptile = pt_pool.tile([P, ICB], bf, tag="pt", name="pt")
                    if r == 0 and jt == 0 and h == 0:
                        # first tile: exp in halves so the first half starts
                        # as soon as the first Q/K projection chunk is done
                        for i5 in range(2):
                            i5s = slice(i5 * 512, (i5 + 1) * 512)
                            nc.scalar.activation(ptile[:, i5s],
                                                 sps_tiles[h][:, i5s],
                                                 AF.Exp, scale=SCALE)
                    else:
                        nc.scalar.activation(ptile[:], sps_tiles[h][:], AF.Exp, scale=SCALE)
                    for sub in range(NSUB):
                        # start=True zeroes the whole 2KB psum bank row, so
                        # only the first matmul into each ob bank may set it
                        first_in_bank = (sub == 0) or (sub == 7 and h == 0)
                        nc.tensor.matmul(
                            ob_slice(ob, h, sub),
                            ptile[:, sub * P:(sub + 1) * P],
                            v_sb[jt][:, hl * (D + 1):(hl + 1) * (D + 1)],
                            start=(jt == 0 and first_in_bank),
                            stop=(jt == NT - 1),
                            skip_group_check=True,
                        )
                sps_tiles = nxt

            # ---- normalize (frees the ob banks for the next round) ----------
            onst = [on_pool.tile([P, P], bf, tag="on", name=f"on{s}")
                    for s in range(NSUB)]
            for h in range(2):
                for sub in range(NSUB):
                    sl = ob_slice(ob, h, sub)
                    rc = rc_pool.tile([P, 1], f32, tag="rc", name="rc")
                    nc.vector.reciprocal(rc[:], sl[:, D:D + 1])
                    dst = onst[sub][:, h * D:(h + 1) * D]
                    if r == 3 and sub % 2 == h:
                        # tail: ACT is idle -> per-partition scale via Copy
                        nc.scalar.activation(dst, sl[:, 0:D], AF.Copy, scale=rc[:])
                    else:
                        nc.vector.tensor_scalar_mul(dst, sl[:, 0:D], rc[:])
            onst_by_round[r] = ((p, icb), onst)

        # ---- tail: last round's transposes via the freed wide ps_s tiles,
        # then icb=1 z chunks rotating through pz + both ps_s bufs ------------
        _, onst3 = onst_by_round[3]
        items = [(h, s) for h in range(2) for s in range(NSUB)]
        for half in range(2):
            tp = ps_s.tile([P, ICB], f32, tag="st", name="tptail")
            grp8 = items[half * 8:(half + 1) * 8]
            for i, (h, sub) in enumerate(grp8):
                nc.tensor.matmul(
                    tp[0:D, i * P:(i + 1) * P],
                    onst3[sub][:, h * D:(h + 1) * D],
                    ident[:],
                    start=(i % 4 == 0),
                    stop=(i % 4 == 3),
                    skip_group_check=True,
                )
            for i, (h, sub) in enumerate(grp8):
                dst = ot[1][h * D:(h + 1) * D,
                            ICB + sub * P: ICB + (sub + 1) * P]
                if i % 2 == 0:
                    nc.scalar.activation(dst, tp[0:D, i * P:(i + 1) * P], AF.Copy)
                else:
                    nc.vector.tensor_copy(dst, tp[0:D, i * P:(i + 1) * P])
        # icb=1 z: one wide psum tile per f-tile; bias-add alternates ACT/DVE
        for ft in range(F // P):
            zp = ps_s.tile([P, ICB], f32, tag="st", name="zptail")
            for ch in range(2):
                isl = slice(ICB + ch * 512, ICB + (ch + 1) * 512)
                for k in range(DH // P):
                    nc.tensor.matmul(
                        zp[:, ch * 512:(ch + 1) * 512],
                        wo[k][:, ft * P:(ft + 1) * P],
                        ot[k][:, isl],
                        start=(k == 0),
                        stop=(k == DH // P - 1),
                    )
            zsb = zs_pool.tile([P, ICB], f32, tag="ztw", name="zsbw", bufs=2)
            if ft % 2 == 0:
                nc.scalar.activation(zsb[:], zp[:], AF.Identity,
                                     bias=zb_sb[:, ft:ft + 1])
            else:
                nc.vector.tensor_scalar_add(zsb[:], zp[:], zb_sb[:, ft:ft + 1])
            nc.sync.dma_start(zt_d[ft * P:(ft + 1) * P, ICB:n], zsb[:])

    nc.compile()
    return nc


def _get_nc():
    if "nc" not in _cache:
        _cache["nc"] = build()
    return _cache["nc"]


def make_in_maps(x, Wq, bq, Wk, bk, Wv, bv, Wo, bo):
    """Host-side sharding: per-core input dict for core c = 2*b + hg."""
    bfnp = _bf_np()
    in_maps = []
    for c in range(8):
        b, hg = divmod(c, 2)
        cs = slice(hg * DH, (hg + 1) * DH)
        wo_s = np.ascontiguousarray(Wo[cs, :])
        zb = np.asarray(bv[cs] @ wo_s, dtype=np.float32)
        in_maps.append({
            "xt": np.ascontiguousarray(np.asarray(x[b]).T),
            "wq": np.ascontiguousarray(Wq[:, cs]),
            "wk": np.ascontiguousarray(Wk[:, cs]),
            "wv": np.ascontiguousarray(Wv[:, cs]),
            "wob": np.ascontiguousarray(wo_s.astype(bfnp)),
            "bq2": np.ascontiguousarray(np.asarray(bq[cs]).reshape(NPAIR, P).T),
            "bk2": np.ascontiguousarray(np.asarray(bk[cs]).reshape(NPAIR, P).T),
            "zb4": np.ascontiguousarray(zb.reshape(F // P, P).T),
            "ident": np.eye(P, dtype=bfnp),
        })
    return in_maps


def kernel(x, Wq, bq, Wk, bk, Wv, bv, Wo, bo):
    from concourse.bass_utils import run_bass_kernel_spmd

    x = np.asarray(x, dtype=np.float32)
    args = [np.asarray(a, dtype=np.float32) for a in (Wq, bq, Wk, bk, Wv, bv, Wo, bo)]
    nc = _get_nc()
    in_maps = make_in_maps(x, *args)
    res = run_bass_kernel_spmd(nc, in_maps, list(range(8)))
    bo = args[-1]
    out = np.empty((B, N, F), dtype=np.float32)
    for b in range(B):
        zt0 = res.results[2 * b]["zt"]
        zt1 = res.results[2 * b + 1]["zt"]
        out[b] = (zt0 + zt1).T + bo
    return out
